# revision 24
# baseline (speedup 1.0000x reference)
"""Trainium2 Bass kernel for KBLAM Gemma3n attention (B=2, S=1024, H=2048,
NH=16, NKV=4, HD=128, KB=1024), sharded over 8 NeuronCores as
(batch x kv-head-group): core = 4*b + g handles batch b and kv head g
(which serves q-heads 4g..4g+3).  Each core computes a partial s-major
output y_part (S, H) = attn_out @ Wo[:, 512g:512g+512].T ; the host sums
the 4 partials per batch.

Design notes (per-phase, tuned against neuron-profile traces):
 - projections, scores and attn@v run bf16 (hidden/weights/q/k/kb
   host-cast or cast on PSUM eviction): same PE column rate as f32r at
   N=512 but half the DMA bytes / SBUF / LDWEIGHTS time.  The rope
   math, softmax reciprocal, normalization and output projection stay
   f32r/f32 for accuracy.
 - startup: xT streams on the scalar HWDGE queue while the k/v/q0
   weight blocks stream on the sync queue; the k, v AND q0 projections
   interleave per h-tile (6 PSUM banks), so the PE is the pacing item
   (1.4us/h-tile vs ~1.0us/h of DMA) from the first tile on.
 - softmax denominator runs on the otherwise-idle Pool engine
   (nc.gpsimd): each exp tile is accumulated into a per-chunk f32r
   SBUF accumulator (tensor_copy + tensor_adds, 427ns each), and ONE
   f32r ones-matmul per chunk folds the partition axis on the PE.
   This replaces the previous hybrid PE/DVE scheme (9 ones-matmuls x
   285ns + DVE chain per chunk = ~2.6us of PE per chunk).
 - exp activations are paired ([128,1024] 2-bank PSUM tiles, one
   ACTIVATE for two score steps, trimmed to the written span) to
   amortize the 352-cycle ACT instruction overhead.
 - chunk pipeline: av matmuls lag two exp-pairs behind the scores and
   carry ACROSS chunk boundaries; each chunk's finish chain (fold ->
   reciprocal -> ones-broadcast -> normalize) is flushed inside the
   NEXT chunk after its pipeline is in flight, so the in-order PE
   queue never head-of-line-blocks on DVE work.
 - the score PSUM pool is shared by the c=1 and c=0 phases (no pool
   transition barrier); the filler pool (psq2) hands its 2 banks to
   the y pool (psy) between the phases.
 - c=1 q-chunks run first; they are ACT-throughput-bound, so the
   qn1-3 projection chunks are interleaved into them as ACT-free PE
   filler (2 h-steps per exp-pair).  The c=1 y tiles (4..7) then feed
   the PE during the c=0 chunks; y(0..3) drain at the end through a
   6-deep PSUM ring with copies and output DMAs alternating
   Vector/Scalar and both DMA queues.
"""
import math
from contextlib import ExitStack

import numpy as np

B, S, H = 2, 1024, 2048
NH, NKV, HD = 16, 4, 128
KB = 1024
THETA = 10000.0
SCALE = 1.0 / math.sqrt(HD)


def _build_program(self_tiles, mixed_idx, n_mask, col0_map):
    """Build the single-core Bass/Tile program."""
    import concourse.tile as tile
    from concourse import bacc, mybir

    f32 = mybir.dt.float32
    f32r = mybir.dt.float32r
    bf16 = mybir.dt.bfloat16
    nc = bacc.Bacc("TRN2", target_bir_lowering=False, debug=False,
                   enable_asserts=False, num_devices=8)

    xT = nc.dram_tensor("xT", [H, S], bf16, kind="ExternalInput")
    # packed weights: per-dt blocks of 16 h-tiles: cols 2048*dt + 128*h
    wq = nc.dram_tensor("wq", [128, 8192], bf16, kind="ExternalInput")
    wqn = nc.dram_tensor("wqn", [128, 8192], bf16, kind="ExternalInput")
    wk = nc.dram_tensor("wk", [128, 2048], bf16, kind="ExternalInput")
    wv = nc.dram_tensor("wv", [128, 2048], bf16, kind="ExternalInput")
    # wo packed: block i at cols 2048*i = Wo_g^T[128i:128i+128, :]
    wo = nc.dram_tensor("wo", [128, 8192], bf16, kind="ExternalInput")
    kbkT = nc.dram_tensor("kbkT", [128, KB], bf16, kind="ExternalInput")
    # kbv packed key-major tiles side by side: tile t at cols 128*t
    kbv = nc.dram_tensor("kbv", [128, KB], bf16, kind="ExternalInput")
    cosT = nc.dram_tensor("cosT", [128, S], bf16, kind="ExternalInput")
    sinT = nc.dram_tensor("sinT", [128, S], bf16, kind="ExternalInput")
    ropePT = nc.dram_tensor("ropePT", [128, 128], bf16, kind="ExternalInput")
    onesb = nc.dram_tensor("onesb", [128, 128], bf16, kind="ExternalInput")
    onesf = nc.dram_tensor("onesf", [128, 128], f32r, kind="ExternalInput")
    identb = nc.dram_tensor("identb", [128, 128], bf16, kind="ExternalInput")
    if n_mask:
        masks = nc.dram_tensor("masks", [128, 512 * n_mask], bf16,
                               kind="ExternalInput")
    # y partials stream out in bf16 (halves the 8MB/core output DMA);
    # the host upcasts and sums the 4 partials per batch in f32.
    y = nc.dram_tensor("y", [S, H], bf16, kind="ExternalOutput")

    with tile.TileContext(nc) as tc, ExitStack() as ctx:
        po = ctx.enter_context(tc.tile_pool(name="projout", bufs=1))
        qTr = po.tile([128, 4096], bf16, tag="qTr")
        qnT = po.tile([128, 4096], bf16, tag="qnT")
        kTr = po.tile([128, 1024], bf16, tag="kTr")
        vkm = po.tile([128, 1024], bf16, tag="vkm")

        consts = ctx.enter_context(tc.tile_pool(name="consts", bufs=1))
        kbp = ctx.enter_context(tc.tile_pool(name="kb", bufs=1))

        # xt and the weight-block ring stay alive through phase 2: the
        # qn1-3 projection chunks run INSIDE the c=1 attention phase as
        # ACT-free PE filler.
        xw = ctx.enter_context(tc.tile_pool(name="xw", bufs=1))
        wpool = ctx.enter_context(tc.tile_pool(name="wt", bufs=4))

        # ---------------- phase 1: projections + rope + v transpose ------
        with tc.tile_pool(name="ptmp", bufs=3) as ptmp, \
             tc.tile_pool(name="psr", bufs=2, space="PSUM") as psr:
            # weights (k/v/q0 interleaved per 4-h group) go on the sync
            # queue; the xT stream has the scalar queue to itself.
            wblk_k = wpool.tile([128, 2048], bf16, tag="wblk", name="wblk_k")
            wblk_v = wpool.tile([128, 2048], bf16, tag="wblk", name="wblk_v")
            wblk_q0 = wpool.tile([128, 2048], bf16, tag="wblk",
                                 name="wblk_q0")
            xt = xw.tile([128, 16384], bf16, tag="xt")
            # first matmul needs wk[:,0:512] (sync) + xt[:,0:512] (scalar);
            # after that the xT stream alternates queues (a single HWDGE
            # queue moves ~128GB/s, not enough for the 1.5us/h-tile PE
            # pace) with the weight pieces squeezed between on sync.
            nc.sync.dma_start(wblk_k[:, 0:512], wk[:, 0:512])
            nc.scalar.dma_start(xt[:, 0:512], xT[0:128, 0:512])
            nc.sync.dma_start(wblk_v[:, 0:512], wv[:, 0:512])
            nc.scalar.dma_start(xt[:, 512:1024], xT[0:128, 512:1024])
            nc.sync.dma_start(wblk_q0[:, 0:512], wq[:, 0:512])

            def xtp(h):
                eng = nc.sync if h % 2 == 1 else nc.scalar
                eng.dma_start(xt[:, 1024 * h:1024 * h + 1024],
                              xT[128 * h:128 * h + 128, :])

            xtp(1), xtp(2)
            nc.sync.dma_start(wblk_k[:, 512:1024], wk[:, 512:1024])
            xtp(3), xtp(4)
            nc.sync.dma_start(wblk_v[:, 512:1024], wv[:, 512:1024])
            nc.scalar.dma_start(wblk_k[:, 1024:1536], wk[:, 1024:1536])
            xtp(5), xtp(6)
            nc.sync.dma_start(wblk_q0[:, 512:1024], wq[:, 512:1024])
            nc.scalar.dma_start(wblk_v[:, 1024:1536], wv[:, 1024:1536])
            xtp(7), xtp(8)
            nc.scalar.dma_start(wblk_q0[:, 1024:1536], wq[:, 1024:1536])
            xtp(9), xtp(10)
            nc.scalar.dma_start(wblk_k[:, 1536:2048], wk[:, 1536:2048])
            xtp(11), xtp(12)
            nc.scalar.dma_start(wblk_v[:, 1536:2048], wv[:, 1536:2048])
            nc.scalar.dma_start(wblk_q0[:, 1536:2048], wq[:, 1536:2048])
            xtp(13), xtp(14), xtp(15)
            rp_sb = consts.tile([128, 128], bf16, tag="rp")
            nc.sync.dma_start(rp_sb[:], ropePT[:])
            id_sb = consts.tile([128, 128], bf16, tag="id")
            nc.sync.dma_start(id_sb[:], identb[:])
            onb_sb = consts.tile([128, 128], bf16, tag="onesb")
            nc.sync.dma_start(onb_sb[:], onesb[:])
            onf_sb = consts.tile([128, 128], f32r, tag="onesf")
            nc.sync.dma_start(onf_sb[:], onesf[:])
            cos_sb = consts.tile([128, S], bf16, tag="cos")
            nc.sync.dma_start(cos_sb[:], cosT[:])
            sin_sb = consts.tile([128, S], bf16, tag="sin")
            nc.sync.dma_start(sin_sb[:], sinT[:])
            vt_tmp = xw.tile([128, 1024], bf16, tag="vt")

            def rope_chunk(ps, half, dst):
                tmp = ptmp.tile([128, 512], bf16, tag="tmp")
                nc.scalar.copy(tmp[:], ps[:])
                pp = psr.tile([128, 512], f32, tag="pp")
                nc.tensor.matmul(pp[:], rp_sb[:], tmp[:], start=True, stop=True)
                cs = cos_sb[:, 512 * half:512 * half + 512]
                sn = sin_sb[:, 512 * half:512 * half + 512]
                t3 = ptmp.tile([128, 512], bf16, tag="t3")
                nc.vector.tensor_mul(t3[:], tmp[:], cs)
                tmp2 = ptmp.tile([128, 512], bf16, tag="tmp2")
                nc.vector.tensor_mul(tmp2[:], pp[:], sn)
                nc.vector.tensor_add(dst, t3[:], tmp2[:])

            # ---- k, v and q0 interleaved per h-tile: rides the xT DMA ----
            with tc.tile_pool(name="pskv", bufs=1, space="PSUM") as pskv:
                pss_k = [pskv.tile([128, 512], f32, tag="pk0", name="pk0"),
                         pskv.tile([128, 512], f32, tag="pk1", name="pk1")]
                pss_v = [pskv.tile([128, 512], f32, tag="pv0", name="pv0"),
                         pskv.tile([128, 512], f32, tag="pv1", name="pv1")]
                pss_q = [pskv.tile([128, 512], f32, tag="pq0", name="pq0"),
                         pskv.tile([128, 512], f32, tag="pq1", name="pq1")]
                for h in range(16):
                    for pss, wblk in ((pss_k, wblk_k), (pss_v, wblk_v),
                                      (pss_q, wblk_q0)):
                        for half in range(2):
                            nc.tensor.matmul(
                                pss[half][:], wblk[:, 128 * h:128 * h + 128],
                                xt[:, 1024 * h + 512 * half:
                                   1024 * h + 512 * half + 512],
                                start=(h == 0), stop=(h == 15))
                for half in range(2):
                    rope_chunk(pss_k[half], half,
                               kTr[:, 512 * half:512 * half + 512])
                for half in range(2):
                    nc.scalar.copy(vt_tmp[:, 512 * half:512 * half + 512],
                                   pss_v[half][:])
                for half in range(2):
                    rope_chunk(pss_q[half], half,
                               qTr[:, 512 * half:512 * half + 512])

            kbk_sb = kbp.tile([128, KB], bf16, tag="kbk")
            kbv_sb = kbp.tile([128, KB], bf16, tag="kbv")
            if n_mask:
                mask_sb = consts.tile([128, 512 * n_mask], bf16, tag="mask")

            # ---- q1..q3 / qn0 chunks (qn0 last: its eviction is a pure
            # scalar copy, so the phase-1 PSUM drain that gates the
            # attention pools is as short as possible).  The v transposes
            # run here (after the kv pool freed its banks) and plug the
            # PE while the q1 weights finish streaming. ----
            with tc.tile_pool(name="ptr", bufs=4, space="PSUM") as ptrp:
                for t in range(8):
                    pst = ptrp.tile([128, 128], bf16, tag="ptr")
                    nc.tensor.transpose(
                        pst[:], vt_tmp[:, 128 * t:128 * t + 128], id_sb[:])
                    ceng = nc.scalar if t % 2 == 0 else nc.vector
                    if t % 2 == 0:
                        ceng.copy(vkm[:, 128 * t:128 * t + 128], pst[:])
                    else:
                        ceng.tensor_copy(vkm[:, 128 * t:128 * t + 128],
                                         pst[:])
            with tc.tile_pool(name="psq", bufs=6, space="PSUM") as psq:
                chunks = [(wq, 1, 'q'), (wq, 2, 'q'), (wq, 3, 'q'),
                          (wqn, 0, 'qn')]
                for ci, (w_dram, dt_i, kind) in enumerate(chunks):
                    wblk = wpool.tile([128, 2048], bf16, tag="wblk",
                                      name="wblk")
                    nc.sync.dma_start(
                        wblk[:], w_dram[:, 2048 * dt_i:2048 * dt_i + 2048])
                    # attention-phase loads interleave on the scalar queue
                    # (idle once the xT stream ends)
                    if ci == 0:
                        nc.scalar.dma_start(kbk_sb[:], kbkT[:])
                        nc.scalar.dma_start(kbv_sb[:], kbv[:])
                    elif ci == 1 and n_mask:
                        nc.scalar.dma_start(mask_sb[:], masks[:])
                    pss = [psq.tile([128, 512], f32, tag="pq", name="pq0"),
                           psq.tile([128, 512], f32, tag="pq", name="pq1")]
                    for h in range(16):
                        for half in range(2):
                            nc.tensor.matmul(
                                pss[half][:], wblk[:, 128 * h:128 * h + 128],
                                xt[:, 1024 * h + 512 * half:
                                   1024 * h + 512 * half + 512],
                                start=(h == 0), stop=(h == 15))
                    for half in range(2):
                        if kind == 'q':
                            dst = qTr[:, 1024 * dt_i + 512 * half:
                                      1024 * dt_i + 512 * half + 512]
                            rope_chunk(pss[half], half, dst)
                        else:
                            nc.scalar.copy(
                                qnT[:, 1024 * dt_i + 512 * half:
                                    1024 * dt_i + 512 * half + 512],
                                pss[half][:])

        # ---------------- phase 2: attention ------------------------------
        onp = ctx.enter_context(tc.tile_pool(name="onp", bufs=1))
        outn = onp.tile([128, 4096], bf16, tag="outn")
        wo_sb = onp.tile([128, 8192], bf16, tag="wo")
        # split across both queues; needed only when y emission starts
        nc.sync.dma_start(wo_sb[:, 0:4096], wo[:, 0:4096])
        nc.scalar.dma_start(wo_sb[:, 4096:8192], wo[:, 4096:8192])

        with tc.tile_pool(name="at", bufs=12) as atp, \
             tc.tile_pool(name="nrm", bufs=2) as nrm, \
             tc.tile_pool(name="psaux", bufs=1, space="PSUM") as psaux, \
             tc.tile_pool(name="psout", bufs=1, space="PSUM") as psout, \
             tc.tile_pool(name="ysb", bufs=4) as ysbp:

            def emit_y_tile(st, psy, tail):
                cy, off = st // 4, 128 * (st % 4)
                ysb = ysbp.tile([128, 2048], bf16, tag="ysb", name="ysb")
                for n in range(4):
                    py = psy.tile([128, 512], f32, tag="y", name="py")
                    for i in range(4):
                        lcol = 1024 * i + 512 * cy + off
                        nc.tensor.matmul(
                            py[:], outn[:, lcol:lcol + 128],
                            wo_sb[:, 2048 * i + 512 * n:
                                  2048 * i + 512 * n + 512],
                            start=(i == 0), stop=(i == 3))
                    if tail and n % 2 == 1:
                        nc.vector.tensor_copy(
                            ysb[:, 512 * n:512 * n + 512], py[:])
                    else:
                        nc.scalar.copy(ysb[:, 512 * n:512 * n + 512],
                                       py[:])
                    if tail and st == 3 and n >= 2:
                        # very last piece: split across both queues so the
                        # post-copy drain is halved
                        for sub in range(2):
                            deng = nc.scalar if (n + sub) % 2 == 0 else \
                                nc.sync
                            col = 512 * n + 256 * sub
                            deng.dma_start(
                                y[128 * st:128 * st + 128, col:col + 256],
                                ysb[:, col:col + 256])
                    else:
                        deng = (nc.scalar if n % 2 == 0 else nc.sync) \
                            if tail else nc.sync
                        deng.dma_start(
                            y[128 * st:128 * st + 128,
                              512 * n:512 * n + 512],
                            ysb[:, 512 * n:512 * n + 512])

            # chunk-finish state carried into the NEXT chunk.  The finish
            # chain is PE-light: one fold ones-matmul, then reciprocal
            # (DVE) -> partition_broadcast (Pool) -> normalize-mul (DVE).
            # It is issued at p==0 of the next chunk; the dependent y-tile
            # matmuls are issued at p==3, by which time the chain is done,
            # so the in-order PE queue never waits on it.
            pending = []
            folded = []
            y_pend = []

            def flush_fold():
                # stage 1 (issued at p==0): fold the denominator
                # accumulator with a single ones-matmul (its input is
                # ready, so the PE never stalls), then reciprocal on the
                # DVE and the f32r cast.
                if not pending:
                    return
                st = pending.pop()
                nc.tensor.matmul(st['aux'][0:1, 0:512], onb_sb[:, 0:1],
                                 st['acc'][:], start=True, stop=True)
                st['rec32'] = nrm.tile([1, 512], f32, tag="rec32",
                                       name="rec32")
                nc.vector.reciprocal_approx_fast(st['rec32'][:],
                                                 st['aux'][0:1, :])
                st['rec'] = nrm.tile([1, 512], f32r, tag="rec",
                                     name="rec")
                nc.vector.tensor_copy(st['rec'][:], st['rec32'][:])
                folded.append(st)

            def flush_bcast():
                # stage 2 (issued at p==2, ~2 exp-pairs later): by now the
                # reciprocal is done, so the broadcast matmul doesn't
                # stall the PE; then normalize outn on the DVE.
                if not folded:
                    return
                st = folded.pop()
                nc.tensor.matmul(st['aux'][:, 0:512], onf_sb[0:1, :],
                                 st['rec'][:], start=True, stop=True)
                bc = nrm.tile([128, 512], f32r, tag="bc")
                nc.vector.tensor_copy(bc[:], st['aux'][:])
                nc.vector.tensor_mul(outn[:, st['qcol']:st['qcol'] + 512],
                                     st['ops'][:], bc[:])
                if st['y_st'] is not None:
                    y_pend.append(st['y_st'])

            # av emission lags two exp-pairs behind and carries ACROSS
            # chunk boundaries, so the PE always has score work queued
            # between a chunk's last ACT and the next chunk's first one.
            ready = []

            def emit_avden():
                pair, ats, st = ready.pop(0)
                for slot, (src, t, c0) in enumerate(pair):
                    off = 512 * slot
                    w = 512 - c0
                    vt_l = (kbv_sb if src == 'kb' else
                            vkm)[:, 128 * t:128 * t + 128]
                    nav, nst = st['nav'], st['nst']
                    nc.tensor.matmul(st['ops'][:, c0:512], vt_l,
                                     ats[:, off:off + w],
                                     start=(nav == 0),
                                     stop=(nav == nst - 1))
                    st['nav'] += 1

            def attn_chunk(c, i, pssc, psy, gsizes, tcols, filler=None):
                qcol = 1024 * i + 512 * c
                # masked self tiles first: their post-exp mask-multiply
                # and acc adds then hide behind the later (unmasked)
                # pairs instead of dangling past the chunk end.
                self_order = [t for t in self_tiles[c]
                              if (t, c) in mixed_idx] + \
                             [t for t in self_tiles[c]
                              if (t, c) not in mixed_idx]
                steps = [('kb', t, 0) for t in range(8)] + \
                        [('sf', t, col0_map[(t, c)]) for t in self_order]
                nst = len(steps)
                assert nst == sum(gsizes)
                st = dict(ops=psout.tile([128, 512], f32, tag="out",
                                         name="ops"),
                          aux=psaux.tile([128, 512], f32, tag="aux",
                                         name="aux"),
                          acc=nrm.tile([128, 512], bf16, tag="acc",
                                       name="acc"),
                          qcol=qcol, nav=0, nst=nst, y_st=None)

                s0 = 0
                nacc = 0
                for p, gs in enumerate(gsizes):
                    group = steps[s0:s0 + gs]
                    s0 += gs
                    ps = pssc.tile([128, tcols], f32, tag="pair", name="ps")
                    for slot, (src, t, c0) in enumerate(group):
                        off = 512 * slot
                        w = 512 - c0
                        if src == 'kb':
                            lhsT = kbk_sb[:, 128 * t:128 * t + 128]
                            rhs = qnT[:, qcol:qcol + 512]
                        else:
                            lhsT = kTr[:, 128 * t:128 * t + 128]
                            rhs = qTr[:, qcol + c0:qcol + 512]
                        nc.tensor.matmul(ps[:, off:off + w], lhsT, rhs,
                                         start=True, stop=True)
                    # one ACT for the group, trimmed to the written span
                    span = 512 * (gs - 1) + 512 - group[-1][2]
                    ats = atp.tile([128, tcols], bf16, tag=f"at{tcols}",
                                   name="ats")
                    nc.scalar.activation(
                        ats[:, 0:span], ps[:, 0:span],
                        mybir.ActivationFunctionType.Exp, scale=SCALE)
                    # causal-mask tiles: multiply by the 0/1 mask on the
                    # otherwise-idle Pool engine, post-exp (all-SBUF) --
                    # keeps both the PE and the DVE out of it.  The av
                    # lag and the dep-ordered acc adds absorb the Pool
                    # latency.
                    for slot, (src, t, c0) in enumerate(group):
                        if src == 'sf' and (t, c) in mixed_idx:
                            k = mixed_idx[(t, c)]
                            off = 512 * slot
                            w = 512 - c0
                            meng = nc.gpsimd if slot % 2 == 0 else nc.vector
                            meng.tensor_mul(
                                ats[:, off:off + w], ats[:, off:off + w],
                                mask_sb[:, 512 * k:512 * k + w])
                    # denominator: accumulate the exp tiles into a bf16
                    # SBUF accumulator on the DVE (all-SBUF 2-byte packed
                    # ops hit the 4x perf mode, ~133ns per add)
                    for slot, (src, t, c0) in enumerate(group):
                        off = 512 * slot
                        w = 512 - c0
                        if nacc == 0:
                            nc.vector.tensor_copy(st['acc'][:],
                                                  ats[:, off:off + w])
                        else:
                            nc.vector.tensor_add(
                                st['acc'][:, c0:512], st['acc'][:, c0:512],
                                ats[:, off:off + w])
                        nacc += 1
                    ready.append((group, ats, st))
                    if p == 2:
                        flush_fold()
                    elif p == 4:
                        flush_bcast()
                    elif p == 5 and y_pend and psy is not None:
                        emit_y_tile(y_pend.pop(0), psy, tail=False)
                    if len(ready) > 2:
                        emit_avden()
                    if filler is not None:
                        filler()
                pending.append(st)

            # One score-pair PSUM pool spans both attention phases (same
            # [128,1024] tile shape), so there is no pool-transition
            # barrier between c=1 and c=0.  The filler pool (psq2) hands
            # its 2 banks to the y pool (psy) between the phases.
            with tc.tile_pool(name="pssc", bufs=2, space="PSUM") as pssc:
                with tc.tile_pool(name="psq2", bufs=2, space="PSUM") as psq2:
                    # the qn1-3 projection chunks run here as PE filler:
                    # the c=1 phase is ACT-throughput-bound and its y
                    # tiles are not ready yet, so these 32-matmul
                    # ACT-free chunks plug the PE idle.  Exactly 2
                    # h-steps per exp-pair x 8 pairs = one full chunk per
                    # head; head i computes qn_{i+1}.
                    def make_qn_filler(j):
                        wblk = wpool.tile([128, 2048], bf16, tag="wblk",
                                          name="wblkqn")
                        nc.sync.dma_start(wblk[:],
                                          wqn[:, 2048 * j:2048 * j + 2048])
                        pss = [psq2.tile([128, 512], f32, tag="pq2",
                                         name="pqa"),
                               psq2.tile([128, 512], f32, tag="pq2",
                                         name="pqb")]
                        hh = [0]

                        def emit2():
                            for _ in range(2):
                                h = hh[0]
                                if h >= 16:
                                    return
                                for half in range(2):
                                    nc.tensor.matmul(
                                        pss[half][:],
                                        wblk[:, 128 * h:128 * h + 128],
                                        xt[:, 1024 * h + 512 * half:
                                           1024 * h + 512 * half + 512],
                                        start=(h == 0), stop=(h == 15))
                                hh[0] += 1

                        def finish():
                            for half in range(2):
                                nc.vector.tensor_copy(
                                    qnT[:, 1024 * j + 512 * half:
                                        1024 * j + 512 * half + 512],
                                    pss[half][:])
                        return emit2, finish

                    fillers = [make_qn_filler(j) for j in (1, 2, 3)]
                    for i in range(4):
                        f = fillers[i] if i < 3 else None
                        attn_chunk(1, i, pssc, None, [2] * 8, 1024,
                                   filler=f[0] if f else None)
                        if f is not None:
                            f[1]()

                with tc.tile_pool(name="psy", bufs=2, space="PSUM") as psy:
                    for i in range(4):
                        # ride y tile 4+i (ready once c=1 head 3 is
                        # normed) on the flush that runs inside this chunk
                        pending[-1]['y_st'] = 4 + i
                        attn_chunk(0, i, pssc, psy, [2] * 6, 1024)
                    while ready:
                        emit_avden()
                    # y tile 0: its head-0..2 partial accumulations (into
                    # the now-idle score PSUM tiles) overlap the last
                    # chunk's finish chain; the head-3 finishers run after
                    # the final normalize lands.
                    psA = pssc.tile([128, 1024], f32, tag="pair",
                                    name="yA")
                    psB = pssc.tile([128, 1024], f32, tag="pair",
                                    name="yB")
                    pys = [psA[:, 0:512], psA[:, 512:1024],
                           psB[:, 0:512], psB[:, 512:1024]]
                    for n in range(4):
                        for i in range(3):
                            nc.tensor.matmul(
                                pys[n], outn[:, 1024 * i:1024 * i + 128],
                                wo_sb[:, 2048 * i + 512 * n:
                                      2048 * i + 512 * n + 512],
                                start=(i == 0), stop=False)
                    # flush the last c=0 chunk
                    flush_fold()
                    flush_bcast()
                    ysb0 = ysbp.tile([128, 2048], bf16, tag="ysb",
                                     name="ysb0")
                    for n in range(4):
                        nc.tensor.matmul(
                            pys[n], outn[:, 3072:3072 + 128],
                            wo_sb[:, 6144 + 512 * n:6144 + 512 * n + 512],
                            start=False, stop=True)
                        if n % 2 == 0:
                            nc.scalar.copy(ysb0[:, 512 * n:512 * n + 512],
                                           pys[n])
                        else:
                            nc.vector.tensor_copy(
                                ysb0[:, 512 * n:512 * n + 512], pys[n])
                        deng = nc.scalar if n % 2 == 0 else nc.sync
                        deng.dma_start(y[0:128, 512 * n:512 * n + 512],
                                       ysb0[:, 512 * n:512 * n + 512])

            # tail scope: the score/aux/out banks are dead now, so the
            # remaining y tiles get a 6-deep PSUM ring -- a shallower
            # ring made each group wait for a PSUM->SBUF copy, and the
            # resulting micro-idles p-state-cooled the PE.
            with tc.tile_pool(name="psyt", bufs=6, space="PSUM") as psyt:
                for st in range(1, 4):
                    emit_y_tile(st, psyt, tail=True)

    nc.compile()
    return nc


def kernel(hidden_states, attention_mask, position_ids, kb_keys, kb_values,
           Wq, Wq_new, Wk, Wv, Wo):
    import ml_dtypes
    from concourse.bass_utils import run_bass_kernel_spmd

    bf16 = ml_dtypes.bfloat16
    hidden_states = np.asarray(hidden_states, dtype=np.float32)
    attention_mask = np.asarray(attention_mask, dtype=np.float32)
    position_ids = np.asarray(position_ids)
    kb_keys = np.asarray(kb_keys, dtype=np.float32)
    kb_values = np.asarray(kb_values, dtype=np.float32)
    Wq = np.asarray(Wq, dtype=np.float32)
    Wq_new = np.asarray(Wq_new, dtype=np.float32)
    Wk = np.asarray(Wk, dtype=np.float32)
    Wv = np.asarray(Wv, dtype=np.float32)
    Wo = np.asarray(Wo, dtype=np.float32)

    # ---- host: classify self-attention mask blocks ----
    mask = attention_mask[:, 0]  # (B, S, S) [q, key]
    self_tiles = {}
    mixed = []
    col0_map = {}
    for c in range(2):
        tiles = []
        for t in range(8):
            blk = mask[:, 512 * c:512 * c + 512, 128 * t:128 * t + 128]
            if np.all(blk <= -1e8):
                continue
            tiles.append(t)
            # leading q-columns fully masked in every batch can be skipped
            colmask = np.all(blk <= -1e8, axis=(0, 2))  # (512,) per q-col
            col0 = 0
            while col0 < 512 and colmask[col0]:
                col0 += 1
            col0 = (col0 // 128) * 128  # keep 128-aligned for tidy tiles
            col0_map[(t, c)] = col0
            if np.any(blk[:, col0:, :] < 0):
                mixed.append((t, c))
        self_tiles[c] = tiles
    mixed_idx = {tc_: k for k, tc_ in enumerate(mixed)}
    n_mask = len(mixed)

    nc = _build_program(self_tiles, mixed_idx, n_mask, col0_map)

    # ---- host: shared constant prep ----
    inv_freq = 1.0 / (THETA ** (np.arange(0, HD, 2, dtype=np.float32) / HD))
    P = np.zeros((HD, HD), np.float32)
    for d in range(64):
        P[d, d + 64] = -1.0
        P[d + 64, d] = 1.0
    ropePT = np.ascontiguousarray(P.T).astype(bf16)
    onesb = np.ones((128, 128), bf16)
    onesf = np.ones((128, 128), np.float32)
    identb = np.eye(128, dtype=np.float32).astype(bf16)

    def pack_w(wT, ndt):
        # wT (H, 128*ndt) -> (128, 2048*ndt): tile (dt) block holds 16
        # h-tiles side by side: cols 2048*dt + 128*h = wT[128h:+128, 128dt:+128]
        out = np.empty((128, 2048 * ndt), bf16)
        for dt_i in range(ndt):
            for h in range(16):
                out[:, 2048 * dt_i + 128 * h:2048 * dt_i + 128 * h + 128] = \
                    wT[128 * h:128 * h + 128, 128 * dt_i:128 * dt_i + 128]
        return out

    cosTs, sinTs, maskTs = [], [], []
    for b in range(B):
        freqs = position_ids[b].astype(np.float32)[:, None] * inv_freq[None, :]
        emb = np.concatenate([freqs, freqs], axis=1)  # (S, 128)
        cosTs.append(np.ascontiguousarray(np.cos(emb).T).astype(bf16))
        sinTs.append(np.ascontiguousarray(np.sin(emb).T).astype(bf16))
        if n_mask:
            mt = np.zeros((128, 512 * n_mask), bf16)
            for (t, c), k in mixed_idx.items():
                c0 = col0_map[(t, c)]
                w = 512 - c0
                mt[:, 512 * k:512 * k + w] = \
                    (mask[b, 512 * c + c0:512 * c + 512,
                          128 * t:128 * t + 128].T > -1e8)
            maskTs.append(mt)

    in_maps = []
    for cid in range(8):
        b, g = cid // 4, cid % 4
        kbv_p = np.empty((128, KB), bf16)
        kvb = kb_values[b, :, 128 * g:128 * g + 128].astype(bf16)
        for t in range(8):
            kbv_p[:, 128 * t:128 * t + 128] = kvb[128 * t:128 * t + 128, :]
        wo_p = np.empty((128, 8192), bf16)
        woT = Wo[:, 512 * g:512 * g + 512].T.astype(bf16)
        for i in range(4):
            wo_p[:, 2048 * i:2048 * i + 2048] = woT[128 * i:128 * i + 128, :]
        m = dict(
            xT=np.ascontiguousarray(hidden_states[b].T).astype(bf16),
            wq=pack_w(Wq[512 * g:512 * g + 512, :].T.astype(bf16), 4),
            wqn=pack_w(Wq_new[512 * g:512 * g + 512, :].T.astype(bf16), 4),
            wk=pack_w(Wk[128 * g:128 * g + 128, :].T.astype(bf16), 1),
            wv=pack_w(Wv[128 * g:128 * g + 128, :].T.astype(bf16), 1),
            wo=wo_p,
            kbkT=np.ascontiguousarray(
                kb_keys[b, :, 128 * g:128 * g + 128].T).astype(bf16),
            kbv=kbv_p,
            cosT=cosTs[b], sinT=sinTs[b],
            ropePT=ropePT, onesb=onesb, onesf=onesf, identb=identb,
        )
        if n_mask:
            m['masks'] = maskTs[b]
        in_maps.append(m)

    res = run_bass_kernel_spmd(nc, in_maps, core_ids=list(range(8)))
    if res.exec_time_ns is not None:
        print(f"HW exec time: {res.exec_time_ns} ns")

    out = np.zeros((B, S, H), np.float32)
    for cid in range(8):
        b = cid // 4
        out[b] += res.results[cid]["y"].astype(np.float32)
    return out


# revision 25
# speedup vs baseline: 1.0342x; 1.0342x over previous
"""Trainium2 Bass kernel for KBLAM Gemma3n attention (B=2, S=1024, H=2048,
NH=16, NKV=4, HD=128, KB=1024), sharded over 8 NeuronCores as
(batch x kv-head-group): core = 4*b + g handles batch b and kv head g
(which serves q-heads 4g..4g+3).  Each core computes a partial s-major
output y_part (S, H) = attn_out @ Wo[:, 512g:512g+512].T ; the host sums
the 4 partials per batch.

Design notes (per-phase, tuned against neuron-profile traces):
 - projections, scores and attn@v run bf16 (hidden/weights/q/k/kb
   host-cast or cast on PSUM eviction): same PE column rate as f32r at
   N=512 but half the DMA bytes / SBUF / LDWEIGHTS time.  The rope
   math, softmax reciprocal, normalization and output projection stay
   f32r/f32 for accuracy.
 - startup: xT streams on the scalar HWDGE queue while the k/v/q0
   weight blocks stream on the sync queue; the k, v AND q0 projections
   interleave per h-tile (6 PSUM banks), so the PE is the pacing item
   (1.4us/h-tile vs ~1.0us/h of DMA) from the first tile on.
 - softmax denominator runs on the otherwise-idle Pool engine
   (nc.gpsimd): each exp tile is accumulated into a per-chunk f32r
   SBUF accumulator (tensor_copy + tensor_adds, 427ns each), and ONE
   f32r ones-matmul per chunk folds the partition axis on the PE.
   This replaces the previous hybrid PE/DVE scheme (9 ones-matmuls x
   285ns + DVE chain per chunk = ~2.6us of PE per chunk).
 - exp activations are paired ([128,1024] 2-bank PSUM tiles, one
   ACTIVATE for two score steps, trimmed to the written span) to
   amortize the 352-cycle ACT instruction overhead.
 - chunk pipeline: av matmuls lag two exp-pairs behind the scores and
   carry ACROSS chunk boundaries; each chunk's finish chain (fold ->
   reciprocal -> ones-broadcast -> normalize) is flushed inside the
   NEXT chunk after its pipeline is in flight, so the in-order PE
   queue never head-of-line-blocks on DVE work.
 - the score PSUM pool is shared by the c=1 and c=0 phases (no pool
   transition barrier); the filler pool (psq2) hands its 2 banks to
   the y pool (psy) between the phases.
 - c=1 q-chunks run first; they are ACT-throughput-bound, so the
   qn1-3 projection chunks are interleaved into them as ACT-free PE
   filler (2 h-steps per exp-pair).  The c=1 y tiles (4..7) then feed
   the PE during the c=0 chunks; y(0..3) drain at the end through a
   6-deep PSUM ring with copies and output DMAs alternating
   Vector/Scalar and both DMA queues.
"""
import math
from contextlib import ExitStack

import numpy as np

B, S, H = 2, 1024, 2048
NH, NKV, HD = 16, 4, 128
KB = 1024
THETA = 10000.0
SCALE = 1.0 / math.sqrt(HD)


def _build_program(self_tiles, mixed_idx, n_mask, col0_map):
    """Build the single-core Bass/Tile program."""
    import concourse.tile as tile
    from concourse import bacc, mybir

    f32 = mybir.dt.float32
    f32r = mybir.dt.float32r
    bf16 = mybir.dt.bfloat16
    nc = bacc.Bacc("TRN2", target_bir_lowering=False, debug=False,
                   enable_asserts=False, num_devices=8)

    xT = nc.dram_tensor("xT", [H, S], bf16, kind="ExternalInput")
    # packed weights: per-dt blocks of 16 h-tiles: cols 2048*dt + 128*h
    wq = nc.dram_tensor("wq", [128, 8192], bf16, kind="ExternalInput")
    wqn = nc.dram_tensor("wqn", [128, 8192], bf16, kind="ExternalInput")
    wk = nc.dram_tensor("wk", [128, 2048], bf16, kind="ExternalInput")
    wv = nc.dram_tensor("wv", [128, 2048], bf16, kind="ExternalInput")
    # wo packed: block i at cols 2048*i = Wo_g^T[128i:128i+128, :]
    wo = nc.dram_tensor("wo", [128, 8192], bf16, kind="ExternalInput")
    kbkT = nc.dram_tensor("kbkT", [128, KB], bf16, kind="ExternalInput")
    # kbv packed key-major tiles side by side: tile t at cols 128*t
    kbv = nc.dram_tensor("kbv", [128, KB], bf16, kind="ExternalInput")
    cosT = nc.dram_tensor("cosT", [128, S], bf16, kind="ExternalInput")
    sinT = nc.dram_tensor("sinT", [128, S], bf16, kind="ExternalInput")
    ropePT = nc.dram_tensor("ropePT", [128, 128], bf16, kind="ExternalInput")
    onesb = nc.dram_tensor("onesb", [128, 128], bf16, kind="ExternalInput")
    onesf = nc.dram_tensor("onesf", [128, 128], f32r, kind="ExternalInput")
    identb = nc.dram_tensor("identb", [128, 128], bf16, kind="ExternalInput")
    if n_mask:
        masks = nc.dram_tensor("masks", [128, 512 * n_mask], bf16,
                               kind="ExternalInput")
    # y partials stream out in bf16 (halves the 8MB/core output DMA);
    # the host upcasts and sums the 4 partials per batch in f32.
    y = nc.dram_tensor("y", [S, H], bf16, kind="ExternalOutput")

    with tile.TileContext(nc) as tc, ExitStack() as ctx:
        po = ctx.enter_context(tc.tile_pool(name="projout", bufs=1))
        qTr = po.tile([128, 4096], bf16, tag="qTr")
        qnT = po.tile([128, 4096], bf16, tag="qnT")
        kTr = po.tile([128, 1024], bf16, tag="kTr")
        vkm = po.tile([128, 1024], bf16, tag="vkm")

        consts = ctx.enter_context(tc.tile_pool(name="consts", bufs=1))
        kbp = ctx.enter_context(tc.tile_pool(name="kb", bufs=1))

        # xt and the weight-block ring stay alive through phase 2: the
        # qn1-3 projection chunks run INSIDE the c=1 attention phase as
        # ACT-free PE filler.
        xw = ctx.enter_context(tc.tile_pool(name="xw", bufs=1))
        wpool = ctx.enter_context(tc.tile_pool(name="wt", bufs=4))

        # ---------------- phase 1: projections + rope + v transpose ------
        with tc.tile_pool(name="ptmp", bufs=3) as ptmp, \
             tc.tile_pool(name="psr", bufs=2, space="PSUM") as psr:
            # weights (k/v/q0 interleaved per 4-h group) go on the sync
            # queue; the xT stream has the scalar queue to itself.
            wblk_k = wpool.tile([128, 2048], bf16, tag="wblk", name="wblk_k")
            wblk_v = wpool.tile([128, 2048], bf16, tag="wblk", name="wblk_v")
            wblk_q0 = wpool.tile([128, 2048], bf16, tag="wblk",
                                 name="wblk_q0")
            xt = xw.tile([128, 16384], bf16, tag="xt")
            # first matmul needs wk[:,0:512] (sync) + xt[:,0:512] (scalar);
            # after that the xT stream alternates queues (a single HWDGE
            # queue moves ~128GB/s, not enough for the 1.5us/h-tile PE
            # pace) with the weight pieces squeezed between on sync.
            nc.sync.dma_start(wblk_k[:, 0:512], wk[:, 0:512])
            nc.scalar.dma_start(xt[:, 0:512], xT[0:128, 0:512])
            nc.sync.dma_start(wblk_v[:, 0:512], wv[:, 0:512])
            nc.scalar.dma_start(xt[:, 512:1024], xT[0:128, 512:1024])
            nc.sync.dma_start(wblk_q0[:, 0:512], wq[:, 0:512])

            def xtp(h):
                eng = nc.sync if h % 2 == 1 else nc.scalar
                eng.dma_start(xt[:, 1024 * h:1024 * h + 1024],
                              xT[128 * h:128 * h + 128, :])

            xtp(1), xtp(2)
            nc.sync.dma_start(wblk_k[:, 512:1024], wk[:, 512:1024])
            xtp(3), xtp(4)
            nc.sync.dma_start(wblk_v[:, 512:1024], wv[:, 512:1024])
            nc.scalar.dma_start(wblk_k[:, 1024:1536], wk[:, 1024:1536])
            xtp(5), xtp(6)
            nc.sync.dma_start(wblk_q0[:, 512:1024], wq[:, 512:1024])
            nc.scalar.dma_start(wblk_v[:, 1024:1536], wv[:, 1024:1536])
            xtp(7), xtp(8)
            nc.scalar.dma_start(wblk_q0[:, 1024:1536], wq[:, 1024:1536])
            xtp(9), xtp(10)
            nc.scalar.dma_start(wblk_k[:, 1536:2048], wk[:, 1536:2048])
            xtp(11), xtp(12)
            nc.scalar.dma_start(wblk_v[:, 1536:2048], wv[:, 1536:2048])
            nc.scalar.dma_start(wblk_q0[:, 1536:2048], wq[:, 1536:2048])
            xtp(13), xtp(14), xtp(15)
            rp_sb = consts.tile([128, 128], bf16, tag="rp")
            nc.sync.dma_start(rp_sb[:], ropePT[:])
            id_sb = consts.tile([128, 128], bf16, tag="id")
            nc.sync.dma_start(id_sb[:], identb[:])
            onb_sb = consts.tile([128, 128], bf16, tag="onesb")
            nc.sync.dma_start(onb_sb[:], onesb[:])
            onf_sb = consts.tile([128, 128], f32r, tag="onesf")
            nc.sync.dma_start(onf_sb[:], onesf[:])
            cos_sb = consts.tile([128, S], bf16, tag="cos")
            nc.sync.dma_start(cos_sb[:], cosT[:])
            sin_sb = consts.tile([128, S], bf16, tag="sin")
            nc.sync.dma_start(sin_sb[:], sinT[:])
            vt_tmp = xw.tile([128, 1024], bf16, tag="vt")

            def rope_chunk(ps, half, dst):
                tmp = ptmp.tile([128, 512], bf16, tag="tmp")
                nc.scalar.copy(tmp[:], ps[:])
                pp = psr.tile([128, 512], f32, tag="pp")
                nc.tensor.matmul(pp[:], rp_sb[:], tmp[:], start=True, stop=True)
                cs = cos_sb[:, 512 * half:512 * half + 512]
                sn = sin_sb[:, 512 * half:512 * half + 512]
                t3 = ptmp.tile([128, 512], bf16, tag="t3")
                nc.vector.tensor_mul(t3[:], tmp[:], cs)
                tmp2 = ptmp.tile([128, 512], bf16, tag="tmp2")
                nc.vector.tensor_mul(tmp2[:], pp[:], sn)
                nc.vector.tensor_add(dst, t3[:], tmp2[:])

            # ---- k, v and q0 interleaved per h-tile: rides the xT DMA ----
            with tc.tile_pool(name="pskv", bufs=1, space="PSUM") as pskv:
                pss_k = [pskv.tile([128, 512], f32, tag="pk0", name="pk0"),
                         pskv.tile([128, 512], f32, tag="pk1", name="pk1")]
                pss_v = [pskv.tile([128, 512], f32, tag="pv0", name="pv0"),
                         pskv.tile([128, 512], f32, tag="pv1", name="pv1")]
                pss_q = [pskv.tile([128, 512], f32, tag="pq0", name="pq0"),
                         pskv.tile([128, 512], f32, tag="pq1", name="pq1")]
                for h in range(16):
                    for pss, wblk in ((pss_k, wblk_k), (pss_v, wblk_v),
                                      (pss_q, wblk_q0)):
                        for half in range(2):
                            nc.tensor.matmul(
                                pss[half][:], wblk[:, 128 * h:128 * h + 128],
                                xt[:, 1024 * h + 512 * half:
                                   1024 * h + 512 * half + 512],
                                start=(h == 0), stop=(h == 15))
                for half in range(2):
                    rope_chunk(pss_k[half], half,
                               kTr[:, 512 * half:512 * half + 512])
                for half in range(2):
                    nc.scalar.copy(vt_tmp[:, 512 * half:512 * half + 512],
                                   pss_v[half][:])
                for half in range(2):
                    rope_chunk(pss_q[half], half,
                               qTr[:, 512 * half:512 * half + 512])

            kbk_sb = kbp.tile([128, KB], bf16, tag="kbk")
            kbv_sb = kbp.tile([128, KB], bf16, tag="kbv")
            if n_mask:
                mask_sb = consts.tile([128, 512 * n_mask], bf16, tag="mask")

            # ---- q1..q3 / qn0 chunks (qn0 last: its eviction is a pure
            # scalar copy, so the phase-1 PSUM drain that gates the
            # attention pools is as short as possible).  The v transposes
            # run here (after the kv pool freed its banks) and plug the
            # PE while the q1 weights finish streaming. ----
            with tc.tile_pool(name="ptr", bufs=4, space="PSUM") as ptrp:
                for t in range(8):
                    pst = ptrp.tile([128, 128], bf16, tag="ptr")
                    nc.tensor.transpose(
                        pst[:], vt_tmp[:, 128 * t:128 * t + 128], id_sb[:])
                    ceng = nc.scalar if t % 2 == 0 else nc.vector
                    if t % 2 == 0:
                        ceng.copy(vkm[:, 128 * t:128 * t + 128], pst[:])
                    else:
                        ceng.tensor_copy(vkm[:, 128 * t:128 * t + 128],
                                         pst[:])
            with tc.tile_pool(name="psq", bufs=6, space="PSUM") as psq:
                chunks = [(wq, 1, 'q'), (wq, 2, 'q'), (wq, 3, 'q'),
                          (wqn, 0, 'qn')]
                for ci, (w_dram, dt_i, kind) in enumerate(chunks):
                    wblk = wpool.tile([128, 2048], bf16, tag="wblk",
                                      name="wblk")
                    nc.sync.dma_start(
                        wblk[:], w_dram[:, 2048 * dt_i:2048 * dt_i + 2048])
                    # attention-phase loads interleave on the scalar queue
                    # (idle once the xT stream ends)
                    if ci == 0:
                        nc.scalar.dma_start(kbk_sb[:], kbkT[:])
                        nc.scalar.dma_start(kbv_sb[:], kbv[:])
                    elif ci == 1 and n_mask:
                        nc.scalar.dma_start(mask_sb[:], masks[:])
                    pss = [psq.tile([128, 512], f32, tag="pq", name="pq0"),
                           psq.tile([128, 512], f32, tag="pq", name="pq1")]
                    for h in range(16):
                        for half in range(2):
                            nc.tensor.matmul(
                                pss[half][:], wblk[:, 128 * h:128 * h + 128],
                                xt[:, 1024 * h + 512 * half:
                                   1024 * h + 512 * half + 512],
                                start=(h == 0), stop=(h == 15))
                    for half in range(2):
                        if kind == 'q':
                            dst = qTr[:, 1024 * dt_i + 512 * half:
                                      1024 * dt_i + 512 * half + 512]
                            rope_chunk(pss[half], half, dst)
                        else:
                            nc.scalar.copy(
                                qnT[:, 1024 * dt_i + 512 * half:
                                    1024 * dt_i + 512 * half + 512],
                                pss[half][:])

        # ---------------- phase 2: attention ------------------------------
        onp = ctx.enter_context(tc.tile_pool(name="onp", bufs=1))
        outn = onp.tile([128, 4096], bf16, tag="outn")
        wo_sb = onp.tile([128, 8192], bf16, tag="wo")
        # split across both queues; needed only when y emission starts
        nc.sync.dma_start(wo_sb[:, 0:4096], wo[:, 0:4096])
        nc.scalar.dma_start(wo_sb[:, 4096:8192], wo[:, 4096:8192])

        with tc.tile_pool(name="at", bufs=12) as atp, \
             tc.tile_pool(name="nrm", bufs=2) as nrm, \
             tc.tile_pool(name="psaux", bufs=1, space="PSUM") as psaux, \
             tc.tile_pool(name="psout", bufs=1, space="PSUM") as psout, \
             tc.tile_pool(name="ysb", bufs=4) as ysbp:

            def emit_y_tile(st, psy, tail):
                cy, off = st // 4, 128 * (st % 4)
                ysb = ysbp.tile([128, 2048], bf16, tag="ysb", name="ysb")
                for n in range(4):
                    py = psy.tile([128, 512], f32, tag="y", name="py")
                    for i in range(4):
                        lcol = 1024 * i + 512 * cy + off
                        nc.tensor.matmul(
                            py[:], outn[:, lcol:lcol + 128],
                            wo_sb[:, 2048 * i + 512 * n:
                                  2048 * i + 512 * n + 512],
                            start=(i == 0), stop=(i == 3))
                    if tail and n % 2 == 1:
                        nc.vector.tensor_copy(
                            ysb[:, 512 * n:512 * n + 512], py[:])
                    else:
                        nc.scalar.copy(ysb[:, 512 * n:512 * n + 512],
                                       py[:])
                    if tail and st == 3 and n >= 2:
                        # very last piece: split across both queues so the
                        # post-copy drain is halved
                        for sub in range(2):
                            deng = nc.scalar if (n + sub) % 2 == 0 else \
                                nc.sync
                            col = 512 * n + 256 * sub
                            deng.dma_start(
                                y[128 * st:128 * st + 128, col:col + 256],
                                ysb[:, col:col + 256])
                    else:
                        deng = (nc.scalar if n % 2 == 0 else nc.sync) \
                            if tail else nc.sync
                        deng.dma_start(
                            y[128 * st:128 * st + 128,
                              512 * n:512 * n + 512],
                            ysb[:, 512 * n:512 * n + 512])

            # chunk-finish state carried into the NEXT chunk.  The finish
            # chain is PE-light: one fold ones-matmul, then reciprocal
            # (DVE) -> partition_broadcast (Pool) -> normalize-mul (DVE).
            # It is issued at p==0 of the next chunk; the dependent y-tile
            # matmuls are issued at p==3, by which time the chain is done,
            # so the in-order PE queue never waits on it.
            pending = []
            folded = []
            y_pend = []

            def flush_fold():
                # stage 1 (issued at p==0): fold the denominator
                # accumulator with a single ones-matmul (its input is
                # ready, so the PE never stalls), then reciprocal on the
                # DVE and the f32r cast.
                if not pending:
                    return
                st = pending.pop()
                nc.tensor.matmul(st['aux'][0:1, 0:512], onb_sb[:, 0:1],
                                 st['acc'][:], start=True, stop=True)
                st['rec32'] = nrm.tile([1, 512], f32, tag="rec32",
                                       name="rec32")
                nc.vector.reciprocal_approx_fast(st['rec32'][:],
                                                 st['aux'][0:1, :])
                st['rec'] = nrm.tile([1, 512], f32r, tag="rec",
                                     name="rec")
                nc.vector.tensor_copy(st['rec'][:], st['rec32'][:])
                folded.append(st)

            def flush_bcast():
                # stage 2 (issued at p==2, ~2 exp-pairs later): by now the
                # reciprocal is done, so the broadcast matmul doesn't
                # stall the PE; then normalize outn on the DVE.
                if not folded:
                    return
                st = folded.pop()
                nc.tensor.matmul(st['aux'][:, 0:512], onf_sb[0:1, :],
                                 st['rec'][:], start=True, stop=True)
                bc = nrm.tile([128, 512], f32r, tag="bc")
                nc.vector.tensor_copy(bc[:], st['aux'][:])
                nc.vector.tensor_mul(outn[:, st['qcol']:st['qcol'] + 512],
                                     st['ops'][:], bc[:])
                if st['y_st'] is not None:
                    y_pend.append(st['y_st'])

            # av emission lags two exp-pairs behind and carries ACROSS
            # chunk boundaries, so the PE always has score work queued
            # between a chunk's last ACT and the next chunk's first one.
            ready = []

            def emit_avden():
                pair, ats, st = ready.pop(0)
                for slot, (src, t, c0) in enumerate(pair):
                    off = 512 * slot
                    w = 512 - c0
                    vt_l = (kbv_sb if src == 'kb' else
                            vkm)[:, 128 * t:128 * t + 128]
                    nav, nst = st['nav'], st['nst']
                    nc.tensor.matmul(st['ops'][:, c0:512], vt_l,
                                     ats[:, off:off + w],
                                     start=(nav == 0),
                                     stop=(nav == nst - 1))
                    st['nav'] += 1

            def attn_chunk(c, i, pssc, psy, gsizes, tcols, filler=None):
                qcol = 1024 * i + 512 * c
                # masked self tiles first: their post-exp mask-multiply
                # and acc adds then hide behind the later (unmasked)
                # pairs instead of dangling past the chunk end.
                self_order = [t for t in self_tiles[c]
                              if (t, c) in mixed_idx] + \
                             [t for t in self_tiles[c]
                              if (t, c) not in mixed_idx]
                steps = [('kb', t, 0) for t in range(8)] + \
                        [('sf', t, col0_map[(t, c)]) for t in self_order]
                nst = len(steps)
                assert nst == sum(gsizes)
                st = dict(ops=psout.tile([128, 512], f32, tag="out",
                                         name="ops"),
                          aux=psaux.tile([128, 512], f32, tag="aux",
                                         name="aux"),
                          acc=nrm.tile([128, 512], bf16, tag="acc",
                                       name="acc"),
                          qcol=qcol, nav=0, nst=nst, y_st=None)

                s0 = 0
                nacc = 0
                for p, gs in enumerate(gsizes):
                    group = steps[s0:s0 + gs]
                    s0 += gs
                    ps = pssc.tile([128, tcols], f32, tag="pair", name="ps")
                    for slot, (src, t, c0) in enumerate(group):
                        off = 512 * slot
                        w = 512 - c0
                        if src == 'kb':
                            lhsT = kbk_sb[:, 128 * t:128 * t + 128]
                            rhs = qnT[:, qcol:qcol + 512]
                        else:
                            lhsT = kTr[:, 128 * t:128 * t + 128]
                            rhs = qTr[:, qcol + c0:qcol + 512]
                        nc.tensor.matmul(ps[:, off:off + w], lhsT, rhs,
                                         start=True, stop=True)
                    # one ACT for the group, trimmed to the written span
                    span = 512 * (gs - 1) + 512 - group[-1][2]
                    ats = atp.tile([128, tcols], bf16, tag=f"at{tcols}",
                                   name="ats")
                    nc.scalar.activation(
                        ats[:, 0:span], ps[:, 0:span],
                        mybir.ActivationFunctionType.Exp, scale=SCALE)
                    # causal-mask tiles: multiply by the 0/1 mask on the
                    # otherwise-idle Pool engine, post-exp (all-SBUF) --
                    # keeps both the PE and the DVE out of it.  The av
                    # lag and the dep-ordered acc adds absorb the Pool
                    # latency.
                    for slot, (src, t, c0) in enumerate(group):
                        if src == 'sf' and (t, c) in mixed_idx:
                            k = mixed_idx[(t, c)]
                            off = 512 * slot
                            w = 512 - c0
                            meng = nc.gpsimd if slot % 2 == 0 else nc.vector
                            meng.tensor_mul(
                                ats[:, off:off + w], ats[:, off:off + w],
                                mask_sb[:, 512 * k:512 * k + w])
                    # denominator: accumulate the exp tiles into a bf16
                    # SBUF accumulator on the DVE (all-SBUF 2-byte packed
                    # ops hit the 4x perf mode, ~133ns per add)
                    for slot, (src, t, c0) in enumerate(group):
                        off = 512 * slot
                        w = 512 - c0
                        if nacc == 0:
                            nc.vector.tensor_copy(st['acc'][:],
                                                  ats[:, off:off + w])
                        else:
                            nc.vector.tensor_add(
                                st['acc'][:, c0:512], st['acc'][:, c0:512],
                                ats[:, off:off + w])
                        nacc += 1
                    ready.append((group, ats, st))
                    if p == 2:
                        flush_fold()
                    elif p == 4:
                        flush_bcast()
                    elif p == 5 and y_pend and psy is not None:
                        emit_y_tile(y_pend.pop(0), psy, tail=False)
                    if len(ready) > 2:
                        emit_avden()
                    if filler is not None:
                        filler()
                pending.append(st)

            # One score-pair PSUM pool spans both attention phases (same
            # [128,1024] tile shape), so there is no pool-transition
            # barrier between c=1 and c=0.  The filler pool (psq2) hands
            # its 2 banks to the y pool (psy) between the phases.
            with tc.tile_pool(name="pssc", bufs=2, space="PSUM") as pssc:
                with tc.tile_pool(name="psq2", bufs=2, space="PSUM") as psq2:
                    # the qn1-3 projection chunks run here as PE filler:
                    # the c=1 phase is ACT-throughput-bound and its y
                    # tiles are not ready yet, so these 32-matmul
                    # ACT-free chunks plug the PE idle.  Exactly 2
                    # h-steps per exp-pair x 8 pairs = one full chunk per
                    # head; head i computes qn_{i+1}.
                    def make_qn_filler(j):
                        wblk = wpool.tile([128, 2048], bf16, tag="wblk",
                                          name="wblkqn")
                        nc.sync.dma_start(wblk[:],
                                          wqn[:, 2048 * j:2048 * j + 2048])
                        pss = [psq2.tile([128, 512], f32, tag="pq2",
                                         name="pqa"),
                               psq2.tile([128, 512], f32, tag="pq2",
                                         name="pqb")]
                        hh = [0]

                        def emit2():
                            for _ in range(2):
                                h = hh[0]
                                if h >= 16:
                                    return
                                for half in range(2):
                                    nc.tensor.matmul(
                                        pss[half][:],
                                        wblk[:, 128 * h:128 * h + 128],
                                        xt[:, 1024 * h + 512 * half:
                                           1024 * h + 512 * half + 512],
                                        start=(h == 0), stop=(h == 15))
                                hh[0] += 1

                        def finish():
                            for half in range(2):
                                nc.vector.tensor_copy(
                                    qnT[:, 1024 * j + 512 * half:
                                        1024 * j + 512 * half + 512],
                                    pss[half][:])
                        return emit2, finish

                    fillers = [make_qn_filler(j) for j in (1, 2, 3)]
                    for i in range(4):
                        f = fillers[i] if i < 3 else None
                        attn_chunk(1, i, pssc, None, [2] * 8, 1024,
                                   filler=f[0] if f else None)
                        if f is not None:
                            f[1]()

                with tc.tile_pool(name="psy", bufs=2, space="PSUM") as psy:
                    for i in range(4):
                        # ride y tile 4+i (ready once c=1 head 3 is
                        # normed) on the flush that runs inside this chunk
                        pending[-1]['y_st'] = 4 + i
                        attn_chunk(0, i, pssc, psy, [2] * 6, 1024)
                    while ready:
                        emit_avden()
                    # y tile 0: its head-0..2 partial accumulations (into
                    # the now-idle score PSUM tiles) overlap the last
                    # chunk's finish chain; the head-3 finishers run after
                    # the final normalize lands.
                    # fold first: its DVE reciprocal chain runs under
                    # the partial matmuls below
                    flush_fold()
                    psA = pssc.tile([128, 1024], f32, tag="pair",
                                    name="yA")
                    psB = pssc.tile([128, 1024], f32, tag="pair",
                                    name="yB")
                    pys = [psA[:, 0:512], psA[:, 512:1024],
                           psB[:, 0:512], psB[:, 512:1024]]
                    for n in range(4):
                        for i in range(3):
                            nc.tensor.matmul(
                                pys[n], outn[:, 1024 * i:1024 * i + 128],
                                wo_sb[:, 2048 * i + 512 * n:
                                      2048 * i + 512 * n + 512],
                                start=(i == 0), stop=False)
                    flush_bcast()
                    ysb0 = ysbp.tile([128, 2048], bf16, tag="ysb",
                                     name="ysb0")
                    for n in range(4):
                        nc.tensor.matmul(
                            pys[n], outn[:, 3072:3072 + 128],
                            wo_sb[:, 6144 + 512 * n:6144 + 512 * n + 512],
                            start=False, stop=True)
                        if n % 2 == 0:
                            nc.scalar.copy(ysb0[:, 512 * n:512 * n + 512],
                                           pys[n])
                        else:
                            nc.vector.tensor_copy(
                                ysb0[:, 512 * n:512 * n + 512], pys[n])
                        deng = nc.scalar if n % 2 == 0 else nc.sync
                        deng.dma_start(y[0:128, 512 * n:512 * n + 512],
                                       ysb0[:, 512 * n:512 * n + 512])

            # tail scope: the score/aux/out banks are dead now, so the
            # remaining y tiles get a 6-deep PSUM ring -- a shallower
            # ring made each group wait for a PSUM->SBUF copy, and the
            # resulting micro-idles p-state-cooled the PE.
            with tc.tile_pool(name="psyt", bufs=6, space="PSUM") as psyt:
                for st in range(1, 4):
                    emit_y_tile(st, psyt, tail=True)

    nc.compile()
    return nc


def kernel(hidden_states, attention_mask, position_ids, kb_keys, kb_values,
           Wq, Wq_new, Wk, Wv, Wo):
    import ml_dtypes
    from concourse.bass_utils import run_bass_kernel_spmd

    bf16 = ml_dtypes.bfloat16
    hidden_states = np.asarray(hidden_states, dtype=np.float32)
    attention_mask = np.asarray(attention_mask, dtype=np.float32)
    position_ids = np.asarray(position_ids)
    kb_keys = np.asarray(kb_keys, dtype=np.float32)
    kb_values = np.asarray(kb_values, dtype=np.float32)
    Wq = np.asarray(Wq, dtype=np.float32)
    Wq_new = np.asarray(Wq_new, dtype=np.float32)
    Wk = np.asarray(Wk, dtype=np.float32)
    Wv = np.asarray(Wv, dtype=np.float32)
    Wo = np.asarray(Wo, dtype=np.float32)

    # ---- host: classify self-attention mask blocks ----
    mask = attention_mask[:, 0]  # (B, S, S) [q, key]
    self_tiles = {}
    mixed = []
    col0_map = {}
    for c in range(2):
        tiles = []
        for t in range(8):
            blk = mask[:, 512 * c:512 * c + 512, 128 * t:128 * t + 128]
            if np.all(blk <= -1e8):
                continue
            tiles.append(t)
            # leading q-columns fully masked in every batch can be skipped
            colmask = np.all(blk <= -1e8, axis=(0, 2))  # (512,) per q-col
            col0 = 0
            while col0 < 512 and colmask[col0]:
                col0 += 1
            col0 = (col0 // 128) * 128  # keep 128-aligned for tidy tiles
            col0_map[(t, c)] = col0
            if np.any(blk[:, col0:, :] < 0):
                mixed.append((t, c))
        self_tiles[c] = tiles
    mixed_idx = {tc_: k for k, tc_ in enumerate(mixed)}
    n_mask = len(mixed)

    nc = _build_program(self_tiles, mixed_idx, n_mask, col0_map)

    # ---- host: shared constant prep ----
    inv_freq = 1.0 / (THETA ** (np.arange(0, HD, 2, dtype=np.float32) / HD))
    P = np.zeros((HD, HD), np.float32)
    for d in range(64):
        P[d, d + 64] = -1.0
        P[d + 64, d] = 1.0
    ropePT = np.ascontiguousarray(P.T).astype(bf16)
    onesb = np.ones((128, 128), bf16)
    onesf = np.ones((128, 128), np.float32)
    identb = np.eye(128, dtype=np.float32).astype(bf16)

    def pack_w(wT, ndt):
        # wT (H, 128*ndt) -> (128, 2048*ndt): tile (dt) block holds 16
        # h-tiles side by side: cols 2048*dt + 128*h = wT[128h:+128, 128dt:+128]
        out = np.empty((128, 2048 * ndt), bf16)
        for dt_i in range(ndt):
            for h in range(16):
                out[:, 2048 * dt_i + 128 * h:2048 * dt_i + 128 * h + 128] = \
                    wT[128 * h:128 * h + 128, 128 * dt_i:128 * dt_i + 128]
        return out

    cosTs, sinTs, maskTs = [], [], []
    for b in range(B):
        freqs = position_ids[b].astype(np.float32)[:, None] * inv_freq[None, :]
        emb = np.concatenate([freqs, freqs], axis=1)  # (S, 128)
        cosTs.append(np.ascontiguousarray(np.cos(emb).T).astype(bf16))
        sinTs.append(np.ascontiguousarray(np.sin(emb).T).astype(bf16))
        if n_mask:
            mt = np.zeros((128, 512 * n_mask), bf16)
            for (t, c), k in mixed_idx.items():
                c0 = col0_map[(t, c)]
                w = 512 - c0
                mt[:, 512 * k:512 * k + w] = \
                    (mask[b, 512 * c + c0:512 * c + 512,
                          128 * t:128 * t + 128].T > -1e8)
            maskTs.append(mt)

    in_maps = []
    for cid in range(8):
        b, g = cid // 4, cid % 4
        kbv_p = np.empty((128, KB), bf16)
        kvb = kb_values[b, :, 128 * g:128 * g + 128].astype(bf16)
        for t in range(8):
            kbv_p[:, 128 * t:128 * t + 128] = kvb[128 * t:128 * t + 128, :]
        wo_p = np.empty((128, 8192), bf16)
        woT = Wo[:, 512 * g:512 * g + 512].T.astype(bf16)
        for i in range(4):
            wo_p[:, 2048 * i:2048 * i + 2048] = woT[128 * i:128 * i + 128, :]
        m = dict(
            xT=np.ascontiguousarray(hidden_states[b].T).astype(bf16),
            wq=pack_w(Wq[512 * g:512 * g + 512, :].T.astype(bf16), 4),
            wqn=pack_w(Wq_new[512 * g:512 * g + 512, :].T.astype(bf16), 4),
            wk=pack_w(Wk[128 * g:128 * g + 128, :].T.astype(bf16), 1),
            wv=pack_w(Wv[128 * g:128 * g + 128, :].T.astype(bf16), 1),
            wo=wo_p,
            kbkT=np.ascontiguousarray(
                kb_keys[b, :, 128 * g:128 * g + 128].T).astype(bf16),
            kbv=kbv_p,
            cosT=cosTs[b], sinT=sinTs[b],
            ropePT=ropePT, onesb=onesb, onesf=onesf, identb=identb,
        )
        if n_mask:
            m['masks'] = maskTs[b]
        in_maps.append(m)

    res = run_bass_kernel_spmd(nc, in_maps, core_ids=list(range(8)))
    if res.exec_time_ns is not None:
        print(f"HW exec time: {res.exec_time_ns} ns")

    out = np.zeros((B, S, H), np.float32)
    for cid in range(8):
        b = cid // 4
        out[b] += res.results[cid]["y"].astype(np.float32)
    return out


# revision 26
# speedup vs baseline: 1.0549x; 1.0201x over previous
"""Trainium2 Bass kernel for KBLAM Gemma3n attention (B=2, S=1024, H=2048,
NH=16, NKV=4, HD=128, KB=1024), sharded over 8 NeuronCores as
(batch x kv-head-group): core = 4*b + g handles batch b and kv head g
(which serves q-heads 4g..4g+3).  Each core computes a partial s-major
output y_part (S, H) = attn_out @ Wo[:, 512g:512g+512].T ; the host sums
the 4 partials per batch.

Design notes (per-phase, tuned against neuron-profile traces):
 - projections, scores and attn@v run bf16 (hidden/weights/q/k/kb
   host-cast or cast on PSUM eviction): same PE column rate as f32r at
   N=512 but half the DMA bytes / SBUF / LDWEIGHTS time.  The rope
   math, softmax reciprocal, normalization and output projection stay
   f32r/f32 for accuracy.
 - startup: xT streams on the scalar HWDGE queue while the k/v/q0
   weight blocks stream on the sync queue; the k, v AND q0 projections
   interleave per h-tile (6 PSUM banks), so the PE is the pacing item
   (1.4us/h-tile vs ~1.0us/h of DMA) from the first tile on.
 - softmax denominator runs on the otherwise-idle Pool engine
   (nc.gpsimd): each exp tile is accumulated into a per-chunk f32r
   SBUF accumulator (tensor_copy + tensor_adds, 427ns each), and ONE
   f32r ones-matmul per chunk folds the partition axis on the PE.
   This replaces the previous hybrid PE/DVE scheme (9 ones-matmuls x
   285ns + DVE chain per chunk = ~2.6us of PE per chunk).
 - exp activations are paired ([128,1024] 2-bank PSUM tiles, one
   ACTIVATE for two score steps, trimmed to the written span) to
   amortize the 352-cycle ACT instruction overhead.
 - chunk pipeline: av matmuls lag two exp-pairs behind the scores and
   carry ACROSS chunk boundaries; each chunk's finish chain (fold ->
   reciprocal -> ones-broadcast -> normalize) is flushed inside the
   NEXT chunk after its pipeline is in flight, so the in-order PE
   queue never head-of-line-blocks on DVE work.
 - the score PSUM pool is shared by the c=1 and c=0 phases (no pool
   transition barrier); the filler pool (psq2) hands its 2 banks to
   the y pool (psy) between the phases.
 - c=1 q-chunks run first; they are ACT-throughput-bound, so the
   qn1-3 projection chunks are interleaved into them as ACT-free PE
   filler (2 h-steps per exp-pair).  The c=1 y tiles (4..7) then feed
   the PE during the c=0 chunks; y(0..3) drain at the end through a
   6-deep PSUM ring with copies and output DMAs alternating
   Vector/Scalar and both DMA queues.
"""
import math
from contextlib import ExitStack

import numpy as np

B, S, H = 2, 1024, 2048
NH, NKV, HD = 16, 4, 128
KB = 1024
THETA = 10000.0
SCALE = 1.0 / math.sqrt(HD)


def _build_program(self_tiles, mixed_idx, n_mask, col0_map):
    """Build the single-core Bass/Tile program."""
    import concourse.tile as tile
    from concourse import bacc, mybir

    f32 = mybir.dt.float32
    f32r = mybir.dt.float32r
    bf16 = mybir.dt.bfloat16
    nc = bacc.Bacc("TRN2", target_bir_lowering=False, debug=False,
                   enable_asserts=False, num_devices=8)

    xT = nc.dram_tensor("xT", [H, S], bf16, kind="ExternalInput")
    # packed weights: per-dt blocks of 16 h-tiles: cols 2048*dt + 128*h
    wq = nc.dram_tensor("wq", [128, 8192], bf16, kind="ExternalInput")
    wqn = nc.dram_tensor("wqn", [128, 8192], bf16, kind="ExternalInput")
    wk = nc.dram_tensor("wk", [128, 2048], bf16, kind="ExternalInput")
    wv = nc.dram_tensor("wv", [128, 2048], bf16, kind="ExternalInput")
    # wo packed: block i at cols 2048*i = Wo_g^T[128i:128i+128, :]
    wo = nc.dram_tensor("wo", [128, 8192], bf16, kind="ExternalInput")
    kbkT = nc.dram_tensor("kbkT", [128, KB], bf16, kind="ExternalInput")
    # kbv packed key-major tiles side by side: tile t at cols 128*t
    kbv = nc.dram_tensor("kbv", [128, KB], bf16, kind="ExternalInput")
    cosT = nc.dram_tensor("cosT", [128, S], bf16, kind="ExternalInput")
    sinT = nc.dram_tensor("sinT", [128, S], bf16, kind="ExternalInput")
    ropePT = nc.dram_tensor("ropePT", [128, 128], bf16, kind="ExternalInput")
    onesb = nc.dram_tensor("onesb", [128, 128], bf16, kind="ExternalInput")
    onesf = nc.dram_tensor("onesf", [128, 128], f32r, kind="ExternalInput")
    identb = nc.dram_tensor("identb", [128, 128], bf16, kind="ExternalInput")
    if n_mask:
        masks = nc.dram_tensor("masks", [128, 512 * n_mask], bf16,
                               kind="ExternalInput")
    # y partials stream out in bf16 (halves the 8MB/core output DMA);
    # the host upcasts and sums the 4 partials per batch in f32.
    y = nc.dram_tensor("y", [S, H], bf16, kind="ExternalOutput")

    with tile.TileContext(nc) as tc, ExitStack() as ctx:
        po = ctx.enter_context(tc.tile_pool(name="projout", bufs=1))
        qTr = po.tile([128, 4096], bf16, tag="qTr")
        qnT = po.tile([128, 4096], bf16, tag="qnT")
        kTr = po.tile([128, 1024], bf16, tag="kTr")
        vkm = po.tile([128, 1024], bf16, tag="vkm")

        consts = ctx.enter_context(tc.tile_pool(name="consts", bufs=1))
        kbp = ctx.enter_context(tc.tile_pool(name="kb", bufs=1))

        # xt and the weight-block ring stay alive through phase 2: the
        # qn1-3 projection chunks run INSIDE the c=1 attention phase as
        # ACT-free PE filler.
        xw = ctx.enter_context(tc.tile_pool(name="xw", bufs=1))
        wpool = ctx.enter_context(tc.tile_pool(name="wt", bufs=4))

        # ---------------- phase 1: projections + rope + v transpose ------
        with tc.tile_pool(name="ptmp", bufs=3) as ptmp, \
             tc.tile_pool(name="psr", bufs=2, space="PSUM") as psr:
            # weights (k/v/q0 interleaved per 4-h group) go on the sync
            # queue; the xT stream has the scalar queue to itself.
            wblk_k = wpool.tile([128, 2048], bf16, tag="wblk", name="wblk_k")
            wblk_v = wpool.tile([128, 2048], bf16, tag="wblk", name="wblk_v")
            wblk_q0 = wpool.tile([128, 2048], bf16, tag="wblk",
                                 name="wblk_q0")
            xt = xw.tile([128, 16384], bf16, tag="xt")
            # first matmul needs wk[:,0:512] (sync) + xt[:,0:512] (scalar);
            # after that the xT stream alternates queues (a single HWDGE
            # queue moves ~128GB/s, not enough for the 1.5us/h-tile PE
            # pace) with the weight pieces squeezed between on sync.
            nc.sync.dma_start(wblk_k[:, 0:512], wk[:, 0:512])
            nc.scalar.dma_start(xt[:, 0:512], xT[0:128, 0:512])
            nc.sync.dma_start(wblk_v[:, 0:512], wv[:, 0:512])
            nc.scalar.dma_start(xt[:, 512:1024], xT[0:128, 512:1024])
            nc.sync.dma_start(wblk_q0[:, 0:512], wq[:, 0:512])

            def xtp(h):
                eng = nc.sync if h % 2 == 1 else nc.scalar
                eng.dma_start(xt[:, 1024 * h:1024 * h + 1024],
                              xT[128 * h:128 * h + 128, :])

            xtp(1), xtp(2)
            nc.sync.dma_start(wblk_k[:, 512:1024], wk[:, 512:1024])
            xtp(3), xtp(4)
            nc.sync.dma_start(wblk_v[:, 512:1024], wv[:, 512:1024])
            nc.scalar.dma_start(wblk_k[:, 1024:1536], wk[:, 1024:1536])
            xtp(5), xtp(6)
            nc.sync.dma_start(wblk_q0[:, 512:1024], wq[:, 512:1024])
            nc.scalar.dma_start(wblk_v[:, 1024:1536], wv[:, 1024:1536])
            xtp(7), xtp(8)
            nc.scalar.dma_start(wblk_q0[:, 1024:1536], wq[:, 1024:1536])
            xtp(9), xtp(10)
            nc.scalar.dma_start(wblk_k[:, 1536:2048], wk[:, 1536:2048])
            xtp(11), xtp(12)
            nc.scalar.dma_start(wblk_v[:, 1536:2048], wv[:, 1536:2048])
            nc.scalar.dma_start(wblk_q0[:, 1536:2048], wq[:, 1536:2048])
            xtp(13), xtp(14), xtp(15)
            rp_sb = consts.tile([128, 128], bf16, tag="rp")
            nc.sync.dma_start(rp_sb[:], ropePT[:])
            id_sb = consts.tile([128, 128], bf16, tag="id")
            nc.sync.dma_start(id_sb[:], identb[:])
            onb_sb = consts.tile([128, 128], bf16, tag="onesb")
            nc.sync.dma_start(onb_sb[:], onesb[:])
            onf_sb = consts.tile([128, 128], f32r, tag="onesf")
            nc.sync.dma_start(onf_sb[:], onesf[:])
            cos_sb = consts.tile([128, S], bf16, tag="cos")
            nc.sync.dma_start(cos_sb[:], cosT[:])
            sin_sb = consts.tile([128, S], bf16, tag="sin")
            nc.sync.dma_start(sin_sb[:], sinT[:])
            vt_tmp = xw.tile([128, 1024], bf16, tag="vt")

            def rope_chunk(ps, half, dst):
                tmp = ptmp.tile([128, 512], bf16, tag="tmp")
                nc.scalar.copy(tmp[:], ps[:])
                pp = psr.tile([128, 512], f32, tag="pp")
                nc.tensor.matmul(pp[:], rp_sb[:], tmp[:], start=True, stop=True)
                cs = cos_sb[:, 512 * half:512 * half + 512]
                sn = sin_sb[:, 512 * half:512 * half + 512]
                t3 = ptmp.tile([128, 512], bf16, tag="t3")
                nc.vector.tensor_mul(t3[:], tmp[:], cs)
                tmp2 = ptmp.tile([128, 512], bf16, tag="tmp2")
                nc.vector.tensor_mul(tmp2[:], pp[:], sn)
                nc.vector.tensor_add(dst, t3[:], tmp2[:])

            # ---- k, v and q0 interleaved per h-tile: rides the xT DMA ----
            with tc.tile_pool(name="pskv", bufs=1, space="PSUM") as pskv:
                pss_k = [pskv.tile([128, 512], f32, tag="pk0", name="pk0"),
                         pskv.tile([128, 512], f32, tag="pk1", name="pk1")]
                pss_v = [pskv.tile([128, 512], f32, tag="pv0", name="pv0"),
                         pskv.tile([128, 512], f32, tag="pv1", name="pv1")]
                pss_q = [pskv.tile([128, 512], f32, tag="pq0", name="pq0"),
                         pskv.tile([128, 512], f32, tag="pq1", name="pq1")]
                for h in range(16):
                    for pss, wblk in ((pss_k, wblk_k), (pss_v, wblk_v),
                                      (pss_q, wblk_q0)):
                        for half in range(2):
                            nc.tensor.matmul(
                                pss[half][:], wblk[:, 128 * h:128 * h + 128],
                                xt[:, 1024 * h + 512 * half:
                                   1024 * h + 512 * half + 512],
                                start=(h == 0), stop=(h == 15))
                for half in range(2):
                    rope_chunk(pss_k[half], half,
                               kTr[:, 512 * half:512 * half + 512])
                for half in range(2):
                    nc.scalar.copy(vt_tmp[:, 512 * half:512 * half + 512],
                                   pss_v[half][:])
                for half in range(2):
                    rope_chunk(pss_q[half], half,
                               qTr[:, 512 * half:512 * half + 512])

            kbk_sb = kbp.tile([128, KB], bf16, tag="kbk")
            kbv_sb = kbp.tile([128, KB], bf16, tag="kbv")
            if n_mask:
                mask_sb = consts.tile([128, 512 * n_mask], bf16, tag="mask")

            # ---- q1..q3 / qn0 chunks (qn0 last: its eviction is a pure
            # scalar copy, so the phase-1 PSUM drain that gates the
            # attention pools is as short as possible).  The v transposes
            # run here (after the kv pool freed its banks) and plug the
            # PE while the q1 weights finish streaming. ----
            with tc.tile_pool(name="ptr", bufs=4, space="PSUM") as ptrp:
                for t in range(8):
                    pst = ptrp.tile([128, 128], bf16, tag="ptr")
                    nc.tensor.transpose(
                        pst[:], vt_tmp[:, 128 * t:128 * t + 128], id_sb[:])
                    ceng = nc.scalar if t % 2 == 0 else nc.vector
                    if t % 2 == 0:
                        ceng.copy(vkm[:, 128 * t:128 * t + 128], pst[:])
                    else:
                        ceng.tensor_copy(vkm[:, 128 * t:128 * t + 128],
                                         pst[:])
            with tc.tile_pool(name="psq", bufs=6, space="PSUM") as psq:
                chunks = [(wq, 1, 'q'), (wq, 2, 'q'), (wq, 3, 'q'),
                          (wqn, 0, 'qn')]
                for ci, (w_dram, dt_i, kind) in enumerate(chunks):
                    wblk = wpool.tile([128, 2048], bf16, tag="wblk",
                                      name="wblk")
                    nc.sync.dma_start(
                        wblk[:], w_dram[:, 2048 * dt_i:2048 * dt_i + 2048])
                    # attention-phase loads interleave on the scalar queue
                    # (idle once the xT stream ends)
                    if ci == 0:
                        nc.scalar.dma_start(kbk_sb[:], kbkT[:])
                        nc.scalar.dma_start(kbv_sb[:], kbv[:])
                    elif ci == 1 and n_mask:
                        nc.scalar.dma_start(mask_sb[:], masks[:])
                    pss = [psq.tile([128, 512], f32, tag="pq", name="pq0"),
                           psq.tile([128, 512], f32, tag="pq", name="pq1")]
                    for h in range(16):
                        for half in range(2):
                            nc.tensor.matmul(
                                pss[half][:], wblk[:, 128 * h:128 * h + 128],
                                xt[:, 1024 * h + 512 * half:
                                   1024 * h + 512 * half + 512],
                                start=(h == 0), stop=(h == 15))
                    for half in range(2):
                        if kind == 'q':
                            dst = qTr[:, 1024 * dt_i + 512 * half:
                                      1024 * dt_i + 512 * half + 512]
                            rope_chunk(pss[half], half, dst)
                        else:
                            nc.scalar.copy(
                                qnT[:, 1024 * dt_i + 512 * half:
                                    1024 * dt_i + 512 * half + 512],
                                pss[half][:])

        # ---------------- phase 2: attention ------------------------------
        onp = ctx.enter_context(tc.tile_pool(name="onp", bufs=1))
        outn = onp.tile([128, 4096], bf16, tag="outn")
        wo_sb = onp.tile([128, 8192], bf16, tag="wo")
        # split across both queues; needed only when y emission starts
        nc.sync.dma_start(wo_sb[:, 0:4096], wo[:, 0:4096])
        nc.scalar.dma_start(wo_sb[:, 4096:8192], wo[:, 4096:8192])

        with tc.tile_pool(name="at", bufs=12) as atp, \
             tc.tile_pool(name="nrm", bufs=2) as nrm, \
             tc.tile_pool(name="psaux", bufs=1, space="PSUM") as psaux, \
             tc.tile_pool(name="psout", bufs=1, space="PSUM") as psout, \
             tc.tile_pool(name="ysb", bufs=4) as ysbp:

            def emit_y_tile(st, psy, tail):
                cy, off = st // 4, 128 * (st % 4)
                ysb = ysbp.tile([128, 2048], bf16, tag="ysb", name="ysb")
                for n in range(4):
                    py = psy.tile([128, 512], f32, tag="y", name="py")
                    for i in range(4):
                        lcol = 1024 * i + 512 * cy + off
                        nc.tensor.matmul(
                            py[:], outn[:, lcol:lcol + 128],
                            wo_sb[:, 2048 * i + 512 * n:
                                  2048 * i + 512 * n + 512],
                            start=(i == 0), stop=(i == 3))
                    if tail and n % 2 == 1:
                        nc.vector.tensor_copy(
                            ysb[:, 512 * n:512 * n + 512], py[:])
                    else:
                        nc.scalar.copy(ysb[:, 512 * n:512 * n + 512],
                                       py[:])
                    if tail and st == 3 and n >= 2:
                        # very last piece: split across both queues so the
                        # post-copy drain is halved
                        for sub in range(2):
                            deng = nc.scalar if (n + sub) % 2 == 0 else \
                                nc.sync
                            col = 512 * n + 256 * sub
                            deng.dma_start(
                                y[128 * st:128 * st + 128, col:col + 256],
                                ysb[:, col:col + 256])
                    else:
                        deng = (nc.scalar if n % 2 == 0 else nc.sync) \
                            if tail else nc.sync
                        deng.dma_start(
                            y[128 * st:128 * st + 128,
                              512 * n:512 * n + 512],
                            ysb[:, 512 * n:512 * n + 512])

            # chunk-finish state carried into the NEXT chunk.  The finish
            # chain is PE-light: one fold ones-matmul, then reciprocal
            # (DVE) -> partition_broadcast (Pool) -> normalize-mul (DVE).
            # It is issued at p==0 of the next chunk; the dependent y-tile
            # matmuls are issued at p==3, by which time the chain is done,
            # so the in-order PE queue never waits on it.
            pending = []
            folded = []
            y_pend = []

            def flush_fold():
                # stage 1 (issued at p==0): fold the denominator
                # accumulator with a single ones-matmul (its input is
                # ready, so the PE never stalls), then reciprocal on the
                # DVE and the f32r cast.
                if not pending:
                    return
                st = pending.pop()
                nc.tensor.matmul(st['aux'][0:1, 0:512], onb_sb[:, 0:1],
                                 st['acc'][:], start=True, stop=True)
                st['rec32'] = nrm.tile([1, 512], f32, tag="rec32",
                                       name="rec32")
                nc.vector.reciprocal_approx_fast(st['rec32'][:],
                                                 st['aux'][0:1, :])
                st['rec'] = nrm.tile([1, 512], f32r, tag="rec",
                                     name="rec")
                nc.vector.tensor_copy(st['rec'][:], st['rec32'][:])
                folded.append(st)

            def flush_bcast():
                # stage 2 (issued at p==2, ~2 exp-pairs later): by now the
                # reciprocal is done, so the broadcast matmul doesn't
                # stall the PE; then normalize outn on the DVE.
                if not folded:
                    return
                st = folded.pop()
                nc.tensor.matmul(st['aux'][:, 0:512], onf_sb[0:1, :],
                                 st['rec'][:], start=True, stop=True)
                bc = nrm.tile([128, 512], f32r, tag="bc")
                nc.vector.tensor_copy(bc[:], st['aux'][:])
                nc.vector.tensor_mul(outn[:, st['qcol']:st['qcol'] + 512],
                                     st['ops'][:], bc[:])
                if st['y_st'] is not None:
                    y_pend.append(st['y_st'])

            # av emission lags two exp-pairs behind and carries ACROSS
            # chunk boundaries, so the PE always has score work queued
            # between a chunk's last ACT and the next chunk's first one.
            ready = []

            def emit_avden():
                pair, ats, st = ready.pop(0)
                for slot, (src, t, c0) in enumerate(pair):
                    off = 512 * slot
                    w = 512 - c0
                    vt_l = (kbv_sb if src == 'kb' else
                            vkm)[:, 128 * t:128 * t + 128]
                    nav, nst = st['nav'], st['nst']
                    nc.tensor.matmul(st['ops'][:, c0:512], vt_l,
                                     ats[:, off:off + w],
                                     start=(nav == 0),
                                     stop=(nav == nst - 1))
                    st['nav'] += 1

            def attn_chunk(c, i, pssc, psy, gsizes, tcols, filler=None):
                qcol = 1024 * i + 512 * c
                # masked self tiles first: their post-exp mask-multiply
                # and acc adds then hide behind the later (unmasked)
                # pairs instead of dangling past the chunk end.
                self_order = [t for t in self_tiles[c]
                              if (t, c) in mixed_idx] + \
                             [t for t in self_tiles[c]
                              if (t, c) not in mixed_idx]
                steps = [('kb', t, 0) for t in range(8)] + \
                        [('sf', t, col0_map[(t, c)]) for t in self_order]
                nst = len(steps)
                assert nst == sum(gsizes)
                st = dict(ops=psout.tile([128, 512], f32, tag="out",
                                         name="ops"),
                          aux=psaux.tile([128, 512], f32, tag="aux",
                                         name="aux"),
                          acc=nrm.tile([128, 512], bf16, tag="acc",
                                       name="acc"),
                          qcol=qcol, nav=0, nst=nst, y_st=None)

                s0 = 0
                nacc = 0
                for p, gs in enumerate(gsizes):
                    group = steps[s0:s0 + gs]
                    s0 += gs
                    ps = pssc.tile([128, tcols], f32, tag="pair", name="ps")
                    for slot, (src, t, c0) in enumerate(group):
                        off = 512 * slot
                        w = 512 - c0
                        if src == 'kb':
                            lhsT = kbk_sb[:, 128 * t:128 * t + 128]
                            rhs = qnT[:, qcol:qcol + 512]
                        else:
                            lhsT = kTr[:, 128 * t:128 * t + 128]
                            rhs = qTr[:, qcol + c0:qcol + 512]
                        nc.tensor.matmul(ps[:, off:off + w], lhsT, rhs,
                                         start=True, stop=True)
                    # one ACT for the group, trimmed to the written span
                    span = 512 * (gs - 1) + 512 - group[-1][2]
                    ats = atp.tile([128, tcols], bf16, tag=f"at{tcols}",
                                   name="ats")
                    nc.scalar.activation(
                        ats[:, 0:span], ps[:, 0:span],
                        mybir.ActivationFunctionType.Exp, scale=SCALE)
                    # causal-mask tiles: multiply by the 0/1 mask on the
                    # otherwise-idle Pool engine, post-exp (all-SBUF) --
                    # keeps both the PE and the DVE out of it.  The av
                    # lag and the dep-ordered acc adds absorb the Pool
                    # latency.
                    for slot, (src, t, c0) in enumerate(group):
                        if src == 'sf' and (t, c) in mixed_idx:
                            k = mixed_idx[(t, c)]
                            off = 512 * slot
                            w = 512 - c0
                            meng = nc.gpsimd if slot % 2 == 0 else nc.vector
                            meng.tensor_mul(
                                ats[:, off:off + w], ats[:, off:off + w],
                                mask_sb[:, 512 * k:512 * k + w])
                    # denominator: accumulate the exp tiles into a bf16
                    # SBUF accumulator on the DVE (all-SBUF 2-byte packed
                    # ops hit the 4x perf mode, ~133ns per add)
                    for slot, (src, t, c0) in enumerate(group):
                        off = 512 * slot
                        w = 512 - c0
                        if nacc == 0:
                            nc.vector.tensor_copy(st['acc'][:],
                                                  ats[:, off:off + w])
                        else:
                            nc.vector.tensor_add(
                                st['acc'][:, c0:512], st['acc'][:, c0:512],
                                ats[:, off:off + w])
                        nacc += 1
                    ready.append((group, ats, st))
                    if p == 2:
                        flush_fold()
                    elif p == 4:
                        flush_bcast()
                    elif p == 5 and y_pend and psy is not None:
                        emit_y_tile(y_pend.pop(0), psy, tail=False)
                    if len(ready) > 2:
                        emit_avden()
                    if filler is not None:
                        filler()
                pending.append(st)

            # One score-pair PSUM pool spans both attention phases (same
            # [128,1024] tile shape), so there is no pool-transition
            # barrier between c=1 and c=0.  The filler pool (psq2) hands
            # its 2 banks to the y pool (psy) between the phases.
            with tc.tile_pool(name="pssc", bufs=2, space="PSUM") as pssc:
                with tc.tile_pool(name="psq2", bufs=2, space="PSUM") as psq2:
                    # the qn1-3 projection chunks run here as PE filler:
                    # the c=1 phase is ACT-throughput-bound and its y
                    # tiles are not ready yet, so these 32-matmul
                    # ACT-free chunks plug the PE idle.  Exactly 2
                    # h-steps per exp-pair x 8 pairs = one full chunk per
                    # head; head i computes qn_{i+1}.
                    def make_qn_filler(j):
                        wblk = wpool.tile([128, 2048], bf16, tag="wblk",
                                          name="wblkqn")
                        nc.sync.dma_start(wblk[:],
                                          wqn[:, 2048 * j:2048 * j + 2048])
                        pss = [psq2.tile([128, 512], f32, tag="pq2",
                                         name="pqa"),
                               psq2.tile([128, 512], f32, tag="pq2",
                                         name="pqb")]
                        hh = [0]

                        def emit2():
                            for _ in range(2):
                                h = hh[0]
                                if h >= 16:
                                    return
                                for half in range(2):
                                    nc.tensor.matmul(
                                        pss[half][:],
                                        wblk[:, 128 * h:128 * h + 128],
                                        xt[:, 1024 * h + 512 * half:
                                           1024 * h + 512 * half + 512],
                                        start=(h == 0), stop=(h == 15))
                                hh[0] += 1

                        def finish():
                            for half in range(2):
                                nc.vector.tensor_copy(
                                    qnT[:, 1024 * j + 512 * half:
                                        1024 * j + 512 * half + 512],
                                    pss[half][:])
                        return emit2, finish

                    fillers = [make_qn_filler(j) for j in (1, 2, 3)]
                    for i in range(4):
                        f = fillers[i] if i < 3 else None
                        attn_chunk(1, i, pssc, None, [2] * 8, 1024,
                                   filler=f[0] if f else None)
                        if f is not None:
                            f[1]()

                with tc.tile_pool(name="psy", bufs=2, space="PSUM") as psy:
                    for i in range(4):
                        # ride y tile 4+i (ready once c=1 head 3 is
                        # normed) on the flush that runs inside this chunk
                        pending[-1]['y_st'] = 4 + i
                        attn_chunk(0, i, pssc, psy, [2] * 6, 1024)
                    while ready:
                        emit_avden()
                    # flush the last c=0 chunk
                    flush_fold()
                    flush_bcast()

            # tail scope: the score/aux/out banks are dead now, so the
            # remaining y tiles get a 6-deep PSUM ring -- a shallower
            # ring made each group wait for a PSUM->SBUF copy, and the
            # resulting micro-idles p-state-cooled the PE.
            with tc.tile_pool(name="psyt", bufs=6, space="PSUM") as psyt:
                for st in range(0, 4):
                    emit_y_tile(st, psyt, tail=True)

    nc.compile()
    return nc


def kernel(hidden_states, attention_mask, position_ids, kb_keys, kb_values,
           Wq, Wq_new, Wk, Wv, Wo):
    import ml_dtypes
    from concourse.bass_utils import run_bass_kernel_spmd

    bf16 = ml_dtypes.bfloat16
    hidden_states = np.asarray(hidden_states, dtype=np.float32)
    attention_mask = np.asarray(attention_mask, dtype=np.float32)
    position_ids = np.asarray(position_ids)
    kb_keys = np.asarray(kb_keys, dtype=np.float32)
    kb_values = np.asarray(kb_values, dtype=np.float32)
    Wq = np.asarray(Wq, dtype=np.float32)
    Wq_new = np.asarray(Wq_new, dtype=np.float32)
    Wk = np.asarray(Wk, dtype=np.float32)
    Wv = np.asarray(Wv, dtype=np.float32)
    Wo = np.asarray(Wo, dtype=np.float32)

    # ---- host: classify self-attention mask blocks ----
    mask = attention_mask[:, 0]  # (B, S, S) [q, key]
    self_tiles = {}
    mixed = []
    col0_map = {}
    for c in range(2):
        tiles = []
        for t in range(8):
            blk = mask[:, 512 * c:512 * c + 512, 128 * t:128 * t + 128]
            if np.all(blk <= -1e8):
                continue
            tiles.append(t)
            # leading q-columns fully masked in every batch can be skipped
            colmask = np.all(blk <= -1e8, axis=(0, 2))  # (512,) per q-col
            col0 = 0
            while col0 < 512 and colmask[col0]:
                col0 += 1
            col0 = (col0 // 128) * 128  # keep 128-aligned for tidy tiles
            col0_map[(t, c)] = col0
            if np.any(blk[:, col0:, :] < 0):
                mixed.append((t, c))
        self_tiles[c] = tiles
    mixed_idx = {tc_: k for k, tc_ in enumerate(mixed)}
    n_mask = len(mixed)

    nc = _build_program(self_tiles, mixed_idx, n_mask, col0_map)

    # ---- host: shared constant prep ----
    inv_freq = 1.0 / (THETA ** (np.arange(0, HD, 2, dtype=np.float32) / HD))
    P = np.zeros((HD, HD), np.float32)
    for d in range(64):
        P[d, d + 64] = -1.0
        P[d + 64, d] = 1.0
    ropePT = np.ascontiguousarray(P.T).astype(bf16)
    onesb = np.ones((128, 128), bf16)
    onesf = np.ones((128, 128), np.float32)
    identb = np.eye(128, dtype=np.float32).astype(bf16)

    def pack_w(wT, ndt):
        # wT (H, 128*ndt) -> (128, 2048*ndt): tile (dt) block holds 16
        # h-tiles side by side: cols 2048*dt + 128*h = wT[128h:+128, 128dt:+128]
        out = np.empty((128, 2048 * ndt), bf16)
        for dt_i in range(ndt):
            for h in range(16):
                out[:, 2048 * dt_i + 128 * h:2048 * dt_i + 128 * h + 128] = \
                    wT[128 * h:128 * h + 128, 128 * dt_i:128 * dt_i + 128]
        return out

    cosTs, sinTs, maskTs = [], [], []
    for b in range(B):
        freqs = position_ids[b].astype(np.float32)[:, None] * inv_freq[None, :]
        emb = np.concatenate([freqs, freqs], axis=1)  # (S, 128)
        cosTs.append(np.ascontiguousarray(np.cos(emb).T).astype(bf16))
        sinTs.append(np.ascontiguousarray(np.sin(emb).T).astype(bf16))
        if n_mask:
            mt = np.zeros((128, 512 * n_mask), bf16)
            for (t, c), k in mixed_idx.items():
                c0 = col0_map[(t, c)]
                w = 512 - c0
                mt[:, 512 * k:512 * k + w] = \
                    (mask[b, 512 * c + c0:512 * c + 512,
                          128 * t:128 * t + 128].T > -1e8)
            maskTs.append(mt)

    in_maps = []
    for cid in range(8):
        b, g = cid // 4, cid % 4
        kbv_p = np.empty((128, KB), bf16)
        kvb = kb_values[b, :, 128 * g:128 * g + 128].astype(bf16)
        for t in range(8):
            kbv_p[:, 128 * t:128 * t + 128] = kvb[128 * t:128 * t + 128, :]
        wo_p = np.empty((128, 8192), bf16)
        woT = Wo[:, 512 * g:512 * g + 512].T.astype(bf16)
        for i in range(4):
            wo_p[:, 2048 * i:2048 * i + 2048] = woT[128 * i:128 * i + 128, :]
        m = dict(
            xT=np.ascontiguousarray(hidden_states[b].T).astype(bf16),
            wq=pack_w(Wq[512 * g:512 * g + 512, :].T.astype(bf16), 4),
            wqn=pack_w(Wq_new[512 * g:512 * g + 512, :].T.astype(bf16), 4),
            wk=pack_w(Wk[128 * g:128 * g + 128, :].T.astype(bf16), 1),
            wv=pack_w(Wv[128 * g:128 * g + 128, :].T.astype(bf16), 1),
            wo=wo_p,
            kbkT=np.ascontiguousarray(
                kb_keys[b, :, 128 * g:128 * g + 128].T).astype(bf16),
            kbv=kbv_p,
            cosT=cosTs[b], sinT=sinTs[b],
            ropePT=ropePT, onesb=onesb, onesf=onesf, identb=identb,
        )
        if n_mask:
            m['masks'] = maskTs[b]
        in_maps.append(m)

    res = run_bass_kernel_spmd(nc, in_maps, core_ids=list(range(8)))
    if res.exec_time_ns is not None:
        print(f"HW exec time: {res.exec_time_ns} ns")

    out = np.zeros((B, S, H), np.float32)
    for cid in range(8):
        b = cid // 4
        out[b] += res.results[cid]["y"].astype(np.float32)
    return out


# revision 27
# speedup vs baseline: 1.0909x; 1.0341x over previous
"""Trainium2 Bass kernel for KBLAM Gemma3n attention (B=2, S=1024, H=2048,
NH=16, NKV=4, HD=128, KB=1024), sharded over 8 NeuronCores as
(batch x kv-head-group): core = 4*b + g handles batch b and kv head g
(which serves q-heads 4g..4g+3).  Each core computes a partial s-major
output y_part (S, H) = attn_out @ Wo[:, 512g:512g+512].T ; the host sums
the 4 partials per batch.

Design notes (per-phase, tuned against neuron-profile traces):
 - projections, scores and attn@v run bf16 (hidden/weights/q/k/kb
   host-cast or cast on PSUM eviction): same PE column rate as f32r at
   N=512 but half the DMA bytes / SBUF / LDWEIGHTS time.  The rope
   math, softmax reciprocal, normalization and output projection stay
   f32r/f32 for accuracy.
 - startup: xT streams on the scalar HWDGE queue while the k/v/q0
   weight blocks stream on the sync queue; the k, v AND q0 projections
   interleave per h-tile (6 PSUM banks), so the PE is the pacing item
   (1.4us/h-tile vs ~1.0us/h of DMA) from the first tile on.
 - softmax denominator runs on the otherwise-idle Pool engine
   (nc.gpsimd): each exp tile is accumulated into a per-chunk f32r
   SBUF accumulator (tensor_copy + tensor_adds, 427ns each), and ONE
   f32r ones-matmul per chunk folds the partition axis on the PE.
   This replaces the previous hybrid PE/DVE scheme (9 ones-matmuls x
   285ns + DVE chain per chunk = ~2.6us of PE per chunk).
 - exp activations are paired ([128,1024] 2-bank PSUM tiles, one
   ACTIVATE for two score steps, trimmed to the written span) to
   amortize the 352-cycle ACT instruction overhead.
 - chunk pipeline: av matmuls lag two exp-pairs behind the scores and
   carry ACROSS chunk boundaries; each chunk's finish chain (fold ->
   reciprocal -> ones-broadcast -> normalize) is flushed inside the
   NEXT chunk after its pipeline is in flight, so the in-order PE
   queue never head-of-line-blocks on DVE work.
 - the score PSUM pool is shared by the c=1 and c=0 phases (no pool
   transition barrier); the filler pool (psq2) hands its 2 banks to
   the y pool (psy) between the phases.
 - c=1 q-chunks run first; they are ACT-throughput-bound, so the
   qn1-3 projection chunks are interleaved into them as ACT-free PE
   filler (2 h-steps per exp-pair).  The c=1 y tiles (4..7) then feed
   the PE during the c=0 chunks; y(0..3) drain at the end through a
   6-deep PSUM ring with copies and output DMAs alternating
   Vector/Scalar and both DMA queues.
"""
import math
from contextlib import ExitStack

import numpy as np

B, S, H = 2, 1024, 2048
NH, NKV, HD = 16, 4, 128
KB = 1024
THETA = 10000.0
SCALE = 1.0 / math.sqrt(HD)


def _build_program(self_tiles, mixed_idx, n_mask, col0_map):
    """Build the single-core Bass/Tile program."""
    import concourse.tile as tile
    from concourse import bacc, mybir

    f32 = mybir.dt.float32
    f32r = mybir.dt.float32r
    bf16 = mybir.dt.bfloat16
    nc = bacc.Bacc("TRN2", target_bir_lowering=False, debug=False,
                   enable_asserts=False, num_devices=8)

    xT = nc.dram_tensor("xT", [H, S], bf16, kind="ExternalInput")
    # packed weights: per-dt blocks of 16 h-tiles: cols 2048*dt + 128*h
    wq = nc.dram_tensor("wq", [128, 8192], bf16, kind="ExternalInput")
    wqn = nc.dram_tensor("wqn", [128, 8192], bf16, kind="ExternalInput")
    wk = nc.dram_tensor("wk", [128, 2048], bf16, kind="ExternalInput")
    wv = nc.dram_tensor("wv", [128, 2048], bf16, kind="ExternalInput")
    # wo packed: block i at cols 2048*i = Wo_g^T[128i:128i+128, :]
    wo = nc.dram_tensor("wo", [128, 8192], bf16, kind="ExternalInput")
    kbkT = nc.dram_tensor("kbkT", [128, KB], bf16, kind="ExternalInput")
    # kbv packed key-major tiles side by side: tile t at cols 128*t
    kbv = nc.dram_tensor("kbv", [128, KB], bf16, kind="ExternalInput")
    cosT = nc.dram_tensor("cosT", [128, S], bf16, kind="ExternalInput")
    sinT = nc.dram_tensor("sinT", [128, S], bf16, kind="ExternalInput")
    ropePT = nc.dram_tensor("ropePT", [128, 128], bf16, kind="ExternalInput")
    onesb = nc.dram_tensor("onesb", [128, 128], bf16, kind="ExternalInput")
    onesf = nc.dram_tensor("onesf", [128, 128], f32r, kind="ExternalInput")
    identf = nc.dram_tensor("identf", [128, 128], f32, kind="ExternalInput")
    if n_mask:
        masks = nc.dram_tensor("masks", [128, 512 * n_mask], bf16,
                               kind="ExternalInput")
    # y partials stream out in bf16 (halves the 8MB/core output DMA);
    # the host upcasts and sums the 4 partials per batch in f32.
    y = nc.dram_tensor("y", [S, H], bf16, kind="ExternalOutput")

    with tile.TileContext(nc) as tc, ExitStack() as ctx:
        po = ctx.enter_context(tc.tile_pool(name="projout", bufs=1))
        qTr = po.tile([128, 4096], bf16, tag="qTr")
        qnT = po.tile([128, 4096], bf16, tag="qnT")
        kTr = po.tile([128, 1024], bf16, tag="kTr")
        vkm = po.tile([128, 1024], bf16, tag="vkm")

        consts = ctx.enter_context(tc.tile_pool(name="consts", bufs=1))
        kbp = ctx.enter_context(tc.tile_pool(name="kb", bufs=1))

        # xt and the weight-block ring stay alive through phase 2: the
        # qn1-3 projection chunks run INSIDE the c=1 attention phase as
        # ACT-free PE filler.
        xw = ctx.enter_context(tc.tile_pool(name="xw", bufs=1))
        wpool = ctx.enter_context(tc.tile_pool(name="wt", bufs=4))

        # ---------------- phase 1: projections + rope + v transpose ------
        with tc.tile_pool(name="ptmp", bufs=3) as ptmp, \
             tc.tile_pool(name="psr", bufs=2, space="PSUM") as psr:
            # weights (k/v/q0 interleaved per 4-h group) go on the sync
            # queue; the xT stream has the scalar queue to itself.
            wblk_k = wpool.tile([128, 2048], bf16, tag="wblk", name="wblk_k")
            wblk_v = wpool.tile([128, 2048], bf16, tag="wblk", name="wblk_v")
            wblk_q0 = wpool.tile([128, 2048], bf16, tag="wblk",
                                 name="wblk_q0")
            xt = xw.tile([128, 16384], bf16, tag="xt")
            # first matmul needs wk[:,0:512] (sync) + xt[:,0:512] (scalar);
            # after that the xT stream alternates queues (a single HWDGE
            # queue moves ~128GB/s, not enough for the 1.5us/h-tile PE
            # pace) with the weight pieces squeezed between on sync.
            nc.sync.dma_start(wblk_k[:, 0:512], wk[:, 0:512])
            nc.scalar.dma_start(xt[:, 0:512], xT[0:128, 0:512])
            nc.sync.dma_start(wblk_v[:, 0:512], wv[:, 0:512])
            nc.scalar.dma_start(xt[:, 512:1024], xT[0:128, 512:1024])
            nc.sync.dma_start(wblk_q0[:, 0:512], wq[:, 0:512])

            def xtp(h):
                eng = nc.sync if h % 2 == 1 else nc.scalar
                eng.dma_start(xt[:, 1024 * h:1024 * h + 1024],
                              xT[128 * h:128 * h + 128, :])

            xtp(1), xtp(2)
            nc.sync.dma_start(wblk_k[:, 512:1024], wk[:, 512:1024])
            xtp(3), xtp(4)
            nc.sync.dma_start(wblk_v[:, 512:1024], wv[:, 512:1024])
            nc.scalar.dma_start(wblk_k[:, 1024:1536], wk[:, 1024:1536])
            xtp(5), xtp(6)
            nc.sync.dma_start(wblk_q0[:, 512:1024], wq[:, 512:1024])
            nc.scalar.dma_start(wblk_v[:, 1024:1536], wv[:, 1024:1536])
            xtp(7), xtp(8)
            nc.scalar.dma_start(wblk_q0[:, 1024:1536], wq[:, 1024:1536])
            xtp(9), xtp(10)
            nc.scalar.dma_start(wblk_k[:, 1536:2048], wk[:, 1536:2048])
            xtp(11), xtp(12)
            nc.scalar.dma_start(wblk_v[:, 1536:2048], wv[:, 1536:2048])
            nc.scalar.dma_start(wblk_q0[:, 1536:2048], wq[:, 1536:2048])
            xtp(13), xtp(14), xtp(15)
            rp_sb = consts.tile([128, 128], bf16, tag="rp")
            nc.sync.dma_start(rp_sb[:], ropePT[:])
            id_sb = consts.tile([128, 128], f32, tag="id")
            nc.sync.dma_start(id_sb[:], identf[:])
            onb_sb = consts.tile([128, 128], bf16, tag="onesb")
            nc.sync.dma_start(onb_sb[:], onesb[:])
            onf_sb = consts.tile([128, 128], f32r, tag="onesf")
            nc.sync.dma_start(onf_sb[:], onesf[:])
            cos_sb = consts.tile([128, S], bf16, tag="cos")
            nc.sync.dma_start(cos_sb[:], cosT[:])
            sin_sb = consts.tile([128, S], bf16, tag="sin")
            nc.sync.dma_start(sin_sb[:], sinT[:])
            vt_tmp = xw.tile([128, 1024], f32, tag="vt")

            def rope_chunk(ps, half, dst):
                tmp = ptmp.tile([128, 512], bf16, tag="tmp")
                nc.scalar.copy(tmp[:], ps[:])
                pp = psr.tile([128, 512], f32, tag="pp")
                nc.tensor.matmul(pp[:], rp_sb[:], tmp[:], start=True, stop=True)
                cs = cos_sb[:, 512 * half:512 * half + 512]
                sn = sin_sb[:, 512 * half:512 * half + 512]
                t3 = ptmp.tile([128, 512], bf16, tag="t3")
                nc.vector.tensor_mul(t3[:], tmp[:], cs)
                tmp2 = ptmp.tile([128, 512], bf16, tag="tmp2")
                nc.vector.tensor_mul(tmp2[:], pp[:], sn)
                nc.vector.tensor_add(dst, t3[:], tmp2[:])

            # ---- k, v and q0 interleaved per h-tile: rides the xT DMA ----
            with tc.tile_pool(name="pskv", bufs=1, space="PSUM") as pskv:
                pss_k = [pskv.tile([128, 512], f32, tag="pk0", name="pk0"),
                         pskv.tile([128, 512], f32, tag="pk1", name="pk1")]
                pss_v = [pskv.tile([128, 512], f32, tag="pv0", name="pv0"),
                         pskv.tile([128, 512], f32, tag="pv1", name="pv1")]
                pss_q = [pskv.tile([128, 512], f32, tag="pq0", name="pq0"),
                         pskv.tile([128, 512], f32, tag="pq1", name="pq1")]
                for h in range(16):
                    for pss, wblk in ((pss_k, wblk_k), (pss_v, wblk_v),
                                      (pss_q, wblk_q0)):
                        for half in range(2):
                            nc.tensor.matmul(
                                pss[half][:], wblk[:, 128 * h:128 * h + 128],
                                xt[:, 1024 * h + 512 * half:
                                   1024 * h + 512 * half + 512],
                                start=(h == 0), stop=(h == 15))
                for half in range(2):
                    rope_chunk(pss_k[half], half,
                               kTr[:, 512 * half:512 * half + 512])
                for half in range(2):
                    nc.scalar.copy(vt_tmp[:, 512 * half:512 * half + 512],
                                   pss_v[half][:])
                for half in range(2):
                    rope_chunk(pss_q[half], half,
                               qTr[:, 512 * half:512 * half + 512])

            kbk_sb = kbp.tile([128, KB], bf16, tag="kbk")
            kbv_sb = kbp.tile([128, KB], bf16, tag="kbv")
            if n_mask:
                mask_sb = consts.tile([128, 512 * n_mask], bf16, tag="mask")

            # ---- q1..q3 / qn0 chunks (qn0 last: its eviction is a pure
            # scalar copy, so the phase-1 PSUM drain that gates the
            # attention pools is as short as possible).  The v transposes
            # run here (after the kv pool freed its banks) and plug the
            # PE while the q1 weights finish streaming. ----
            with tc.tile_pool(name="psq", bufs=6, space="PSUM") as psq:
                # v transposes run in f32 out of the q-chunk PSUM ring --
                # no extra pool, so there is no pool-transition barrier
                # between the h-loop drain and the q1 chunk; they fill
                # the PE while the q1 weights finish streaming.
                for t in range(8):
                    pst = psq.tile([128, 512], f32, tag="pq", name="pst")
                    nc.tensor.transpose(
                        pst[:, 0:128], vt_tmp[:, 128 * t:128 * t + 128],
                        id_sb[:])
                    if t % 2 == 0:
                        nc.scalar.copy(vkm[:, 128 * t:128 * t + 128],
                                       pst[:, 0:128])
                    else:
                        nc.vector.tensor_copy(
                            vkm[:, 128 * t:128 * t + 128], pst[:, 0:128])
                chunks = [(wq, 1, 'q'), (wq, 2, 'q'), (wq, 3, 'q'),
                          (wqn, 0, 'qn')]
                for ci, (w_dram, dt_i, kind) in enumerate(chunks):
                    wblk = wpool.tile([128, 2048], bf16, tag="wblk",
                                      name="wblk")
                    nc.sync.dma_start(
                        wblk[:], w_dram[:, 2048 * dt_i:2048 * dt_i + 2048])
                    # attention-phase loads interleave on the scalar queue
                    # (idle once the xT stream ends)
                    if ci == 0:
                        nc.scalar.dma_start(kbk_sb[:], kbkT[:])
                        nc.scalar.dma_start(kbv_sb[:], kbv[:])
                    elif ci == 1 and n_mask:
                        nc.scalar.dma_start(mask_sb[:], masks[:])
                    pss = [psq.tile([128, 512], f32, tag="pq", name="pq0"),
                           psq.tile([128, 512], f32, tag="pq", name="pq1")]
                    for h in range(16):
                        for half in range(2):
                            nc.tensor.matmul(
                                pss[half][:], wblk[:, 128 * h:128 * h + 128],
                                xt[:, 1024 * h + 512 * half:
                                   1024 * h + 512 * half + 512],
                                start=(h == 0), stop=(h == 15))
                    for half in range(2):
                        if kind == 'q':
                            dst = qTr[:, 1024 * dt_i + 512 * half:
                                      1024 * dt_i + 512 * half + 512]
                            rope_chunk(pss[half], half, dst)
                        else:
                            nc.scalar.copy(
                                qnT[:, 1024 * dt_i + 512 * half:
                                    1024 * dt_i + 512 * half + 512],
                                pss[half][:])

        # ---------------- phase 2: attention ------------------------------
        onp = ctx.enter_context(tc.tile_pool(name="onp", bufs=1))
        outn = onp.tile([128, 4096], bf16, tag="outn")
        wo_sb = onp.tile([128, 8192], bf16, tag="wo")
        # split across both queues; needed only when y emission starts
        nc.sync.dma_start(wo_sb[:, 0:4096], wo[:, 0:4096])
        nc.scalar.dma_start(wo_sb[:, 4096:8192], wo[:, 4096:8192])

        with tc.tile_pool(name="at", bufs=12) as atp, \
             tc.tile_pool(name="nrm", bufs=2) as nrm, \
             tc.tile_pool(name="psaux", bufs=1, space="PSUM") as psaux, \
             tc.tile_pool(name="psout", bufs=1, space="PSUM") as psout, \
             tc.tile_pool(name="ysb", bufs=4) as ysbp:

            def emit_y_tile(st, psy, tail):
                cy, off = st // 4, 128 * (st % 4)
                ysb = ysbp.tile([128, 2048], bf16, tag="ysb", name="ysb")
                for n in range(4):
                    py = psy.tile([128, 512], f32, tag="y", name="py")
                    for i in range(4):
                        lcol = 1024 * i + 512 * cy + off
                        nc.tensor.matmul(
                            py[:], outn[:, lcol:lcol + 128],
                            wo_sb[:, 2048 * i + 512 * n:
                                  2048 * i + 512 * n + 512],
                            start=(i == 0), stop=(i == 3))
                    if tail and n % 2 == 1:
                        nc.vector.tensor_copy(
                            ysb[:, 512 * n:512 * n + 512], py[:])
                    else:
                        nc.scalar.copy(ysb[:, 512 * n:512 * n + 512],
                                       py[:])
                    if tail and st == 3 and n >= 2:
                        # very last piece: split across both queues so the
                        # post-copy drain is halved
                        for sub in range(2):
                            deng = nc.scalar if (n + sub) % 2 == 0 else \
                                nc.sync
                            col = 512 * n + 256 * sub
                            deng.dma_start(
                                y[128 * st:128 * st + 128, col:col + 256],
                                ysb[:, col:col + 256])
                    else:
                        deng = (nc.scalar if n % 2 == 0 else nc.sync) \
                            if tail else nc.sync
                        deng.dma_start(
                            y[128 * st:128 * st + 128,
                              512 * n:512 * n + 512],
                            ysb[:, 512 * n:512 * n + 512])

            # chunk-finish state carried into the NEXT chunk.  The finish
            # chain is PE-light: one fold ones-matmul, then reciprocal
            # (DVE) -> partition_broadcast (Pool) -> normalize-mul (DVE).
            # It is issued at p==0 of the next chunk; the dependent y-tile
            # matmuls are issued at p==3, by which time the chain is done,
            # so the in-order PE queue never waits on it.
            pending = []
            folded = []
            y_pend = []

            def flush_fold():
                # stage 1 (issued at p==0): fold the denominator
                # accumulator with a single ones-matmul (its input is
                # ready, so the PE never stalls), then reciprocal on the
                # DVE and the f32r cast.
                if not pending:
                    return
                st = pending.pop()
                nc.tensor.matmul(st['aux'][0:1, 0:512], onb_sb[:, 0:1],
                                 st['acc'][:], start=True, stop=True)
                st['rec32'] = nrm.tile([1, 512], f32, tag="rec32",
                                       name="rec32")
                nc.vector.reciprocal_approx_fast(st['rec32'][:],
                                                 st['aux'][0:1, :])
                st['rec'] = nrm.tile([1, 512], f32r, tag="rec",
                                     name="rec")
                nc.vector.tensor_copy(st['rec'][:], st['rec32'][:])
                folded.append(st)

            def flush_bcast():
                # stage 2 (issued at p==2, ~2 exp-pairs later): by now the
                # reciprocal is done, so the broadcast matmul doesn't
                # stall the PE; then normalize outn on the DVE.
                if not folded:
                    return
                st = folded.pop()
                nc.tensor.matmul(st['aux'][:, 0:512], onf_sb[0:1, :],
                                 st['rec'][:], start=True, stop=True)
                bc = nrm.tile([128, 512], f32r, tag="bc")
                nc.vector.tensor_copy(bc[:], st['aux'][:])
                nc.vector.tensor_mul(outn[:, st['qcol']:st['qcol'] + 512],
                                     st['ops'][:], bc[:])
                if st['y_st'] is not None:
                    y_pend.append(st['y_st'])

            # av emission lags two exp-pairs behind and carries ACROSS
            # chunk boundaries, so the PE always has score work queued
            # between a chunk's last ACT and the next chunk's first one.
            ready = []

            def emit_avden():
                pair, ats, st = ready.pop(0)
                for slot, (src, t, c0) in enumerate(pair):
                    off = 512 * slot
                    w = 512 - c0
                    vt_l = (kbv_sb if src == 'kb' else
                            vkm)[:, 128 * t:128 * t + 128]
                    nav, nst = st['nav'], st['nst']
                    nc.tensor.matmul(st['ops'][:, c0:512], vt_l,
                                     ats[:, off:off + w],
                                     start=(nav == 0),
                                     stop=(nav == nst - 1))
                    st['nav'] += 1

            def attn_chunk(c, i, pssc, psy, gsizes, tcols, filler=None):
                qcol = 1024 * i + 512 * c
                # masked self tiles first: their post-exp mask-multiply
                # and acc adds then hide behind the later (unmasked)
                # pairs instead of dangling past the chunk end.
                self_order = [t for t in self_tiles[c]
                              if (t, c) in mixed_idx] + \
                             [t for t in self_tiles[c]
                              if (t, c) not in mixed_idx]
                steps = [('kb', t, 0) for t in range(8)] + \
                        [('sf', t, col0_map[(t, c)]) for t in self_order]
                nst = len(steps)
                assert nst == sum(gsizes)
                st = dict(ops=psout.tile([128, 512], f32, tag="out",
                                         name="ops"),
                          aux=psaux.tile([128, 512], f32, tag="aux",
                                         name="aux"),
                          acc=nrm.tile([128, 512], bf16, tag="acc",
                                       name="acc"),
                          qcol=qcol, nav=0, nst=nst, y_st=None)

                s0 = 0
                nacc = 0
                for p, gs in enumerate(gsizes):
                    group = steps[s0:s0 + gs]
                    s0 += gs
                    ps = pssc.tile([128, tcols], f32, tag="pair", name="ps")
                    for slot, (src, t, c0) in enumerate(group):
                        off = 512 * slot
                        w = 512 - c0
                        if src == 'kb':
                            lhsT = kbk_sb[:, 128 * t:128 * t + 128]
                            rhs = qnT[:, qcol:qcol + 512]
                        else:
                            lhsT = kTr[:, 128 * t:128 * t + 128]
                            rhs = qTr[:, qcol + c0:qcol + 512]
                        nc.tensor.matmul(ps[:, off:off + w], lhsT, rhs,
                                         start=True, stop=True)
                    # one ACT for the group, trimmed to the written span
                    span = 512 * (gs - 1) + 512 - group[-1][2]
                    ats = atp.tile([128, tcols], bf16, tag=f"at{tcols}",
                                   name="ats")
                    nc.scalar.activation(
                        ats[:, 0:span], ps[:, 0:span],
                        mybir.ActivationFunctionType.Exp, scale=SCALE)
                    # causal-mask tiles: multiply by the 0/1 mask on the
                    # otherwise-idle Pool engine, post-exp (all-SBUF) --
                    # keeps both the PE and the DVE out of it.  The av
                    # lag and the dep-ordered acc adds absorb the Pool
                    # latency.
                    for slot, (src, t, c0) in enumerate(group):
                        if src == 'sf' and (t, c) in mixed_idx:
                            k = mixed_idx[(t, c)]
                            off = 512 * slot
                            w = 512 - c0
                            meng = nc.gpsimd if slot % 2 == 0 else nc.vector
                            meng.tensor_mul(
                                ats[:, off:off + w], ats[:, off:off + w],
                                mask_sb[:, 512 * k:512 * k + w])
                    # denominator: accumulate the exp tiles into a bf16
                    # SBUF accumulator on the DVE (all-SBUF 2-byte packed
                    # ops hit the 4x perf mode, ~133ns per add)
                    for slot, (src, t, c0) in enumerate(group):
                        off = 512 * slot
                        w = 512 - c0
                        if nacc == 0:
                            nc.vector.tensor_copy(st['acc'][:],
                                                  ats[:, off:off + w])
                        else:
                            nc.vector.tensor_add(
                                st['acc'][:, c0:512], st['acc'][:, c0:512],
                                ats[:, off:off + w])
                        nacc += 1
                    ready.append((group, ats, st))
                    if p == 2:
                        flush_fold()
                    elif p == 4:
                        flush_bcast()
                    elif p == 5 and y_pend and psy is not None:
                        emit_y_tile(y_pend.pop(0), psy, tail=False)
                    if len(ready) > 2:
                        emit_avden()
                    if filler is not None:
                        filler()
                pending.append(st)

            # One score-pair PSUM pool spans both attention phases (same
            # [128,1024] tile shape), so there is no pool-transition
            # barrier between c=1 and c=0.  The filler pool (psq2) hands
            # its 2 banks to the y pool (psy) between the phases.
            with tc.tile_pool(name="pssc", bufs=2, space="PSUM") as pssc:
                with tc.tile_pool(name="psq2", bufs=2, space="PSUM") as psq2:
                    # the qn1-3 projection chunks run here as PE filler:
                    # the c=1 phase is ACT-throughput-bound and its y
                    # tiles are not ready yet, so these 32-matmul
                    # ACT-free chunks plug the PE idle.  Exactly 2
                    # h-steps per exp-pair x 8 pairs = one full chunk per
                    # head; head i computes qn_{i+1}.
                    def make_qn_filler(j):
                        wblk = wpool.tile([128, 2048], bf16, tag="wblk",
                                          name="wblkqn")
                        nc.sync.dma_start(wblk[:],
                                          wqn[:, 2048 * j:2048 * j + 2048])
                        pss = [psq2.tile([128, 512], f32, tag="pq2",
                                         name="pqa"),
                               psq2.tile([128, 512], f32, tag="pq2",
                                         name="pqb")]
                        hh = [0]

                        def emit2():
                            for _ in range(2):
                                h = hh[0]
                                if h >= 16:
                                    return
                                for half in range(2):
                                    nc.tensor.matmul(
                                        pss[half][:],
                                        wblk[:, 128 * h:128 * h + 128],
                                        xt[:, 1024 * h + 512 * half:
                                           1024 * h + 512 * half + 512],
                                        start=(h == 0), stop=(h == 15))
                                hh[0] += 1

                        def finish():
                            for half in range(2):
                                nc.vector.tensor_copy(
                                    qnT[:, 1024 * j + 512 * half:
                                        1024 * j + 512 * half + 512],
                                    pss[half][:])
                        return emit2, finish

                    fillers = [make_qn_filler(j) for j in (1, 2, 3)]
                    for i in range(4):
                        f = fillers[i] if i < 3 else None
                        attn_chunk(1, i, pssc, None, [2] * 8, 1024,
                                   filler=f[0] if f else None)
                        if f is not None:
                            f[1]()

                with tc.tile_pool(name="psy", bufs=2, space="PSUM") as psy:
                    for i in range(4):
                        # ride y tile 4+i (ready once c=1 head 3 is
                        # normed) on the flush that runs inside this chunk
                        pending[-1]['y_st'] = 4 + i
                        attn_chunk(0, i, pssc, psy, [2] * 6, 1024)
                    while ready:
                        emit_avden()
                    # flush the last c=0 chunk
                    flush_fold()
                    flush_bcast()

            # tail scope: the score/aux/out banks are dead now, so the
            # remaining y tiles get a 6-deep PSUM ring -- a shallower
            # ring made each group wait for a PSUM->SBUF copy, and the
            # resulting micro-idles p-state-cooled the PE.
            with tc.tile_pool(name="psyt", bufs=6, space="PSUM") as psyt:
                for st in range(0, 4):
                    emit_y_tile(st, psyt, tail=True)

    nc.compile()
    return nc


def kernel(hidden_states, attention_mask, position_ids, kb_keys, kb_values,
           Wq, Wq_new, Wk, Wv, Wo):
    import ml_dtypes
    from concourse.bass_utils import run_bass_kernel_spmd

    bf16 = ml_dtypes.bfloat16
    hidden_states = np.asarray(hidden_states, dtype=np.float32)
    attention_mask = np.asarray(attention_mask, dtype=np.float32)
    position_ids = np.asarray(position_ids)
    kb_keys = np.asarray(kb_keys, dtype=np.float32)
    kb_values = np.asarray(kb_values, dtype=np.float32)
    Wq = np.asarray(Wq, dtype=np.float32)
    Wq_new = np.asarray(Wq_new, dtype=np.float32)
    Wk = np.asarray(Wk, dtype=np.float32)
    Wv = np.asarray(Wv, dtype=np.float32)
    Wo = np.asarray(Wo, dtype=np.float32)

    # ---- host: classify self-attention mask blocks ----
    mask = attention_mask[:, 0]  # (B, S, S) [q, key]
    self_tiles = {}
    mixed = []
    col0_map = {}
    for c in range(2):
        tiles = []
        for t in range(8):
            blk = mask[:, 512 * c:512 * c + 512, 128 * t:128 * t + 128]
            if np.all(blk <= -1e8):
                continue
            tiles.append(t)
            # leading q-columns fully masked in every batch can be skipped
            colmask = np.all(blk <= -1e8, axis=(0, 2))  # (512,) per q-col
            col0 = 0
            while col0 < 512 and colmask[col0]:
                col0 += 1
            col0 = (col0 // 128) * 128  # keep 128-aligned for tidy tiles
            col0_map[(t, c)] = col0
            if np.any(blk[:, col0:, :] < 0):
                mixed.append((t, c))
        self_tiles[c] = tiles
    mixed_idx = {tc_: k for k, tc_ in enumerate(mixed)}
    n_mask = len(mixed)

    nc = _build_program(self_tiles, mixed_idx, n_mask, col0_map)

    # ---- host: shared constant prep ----
    inv_freq = 1.0 / (THETA ** (np.arange(0, HD, 2, dtype=np.float32) / HD))
    P = np.zeros((HD, HD), np.float32)
    for d in range(64):
        P[d, d + 64] = -1.0
        P[d + 64, d] = 1.0
    ropePT = np.ascontiguousarray(P.T).astype(bf16)
    onesb = np.ones((128, 128), bf16)
    onesf = np.ones((128, 128), np.float32)
    identf = np.eye(128, dtype=np.float32)

    def pack_w(wT, ndt):
        # wT (H, 128*ndt) -> (128, 2048*ndt): tile (dt) block holds 16
        # h-tiles side by side: cols 2048*dt + 128*h = wT[128h:+128, 128dt:+128]
        out = np.empty((128, 2048 * ndt), bf16)
        for dt_i in range(ndt):
            for h in range(16):
                out[:, 2048 * dt_i + 128 * h:2048 * dt_i + 128 * h + 128] = \
                    wT[128 * h:128 * h + 128, 128 * dt_i:128 * dt_i + 128]
        return out

    cosTs, sinTs, maskTs = [], [], []
    for b in range(B):
        freqs = position_ids[b].astype(np.float32)[:, None] * inv_freq[None, :]
        emb = np.concatenate([freqs, freqs], axis=1)  # (S, 128)
        cosTs.append(np.ascontiguousarray(np.cos(emb).T).astype(bf16))
        sinTs.append(np.ascontiguousarray(np.sin(emb).T).astype(bf16))
        if n_mask:
            mt = np.zeros((128, 512 * n_mask), bf16)
            for (t, c), k in mixed_idx.items():
                c0 = col0_map[(t, c)]
                w = 512 - c0
                mt[:, 512 * k:512 * k + w] = \
                    (mask[b, 512 * c + c0:512 * c + 512,
                          128 * t:128 * t + 128].T > -1e8)
            maskTs.append(mt)

    in_maps = []
    for cid in range(8):
        b, g = cid // 4, cid % 4
        kbv_p = np.empty((128, KB), bf16)
        kvb = kb_values[b, :, 128 * g:128 * g + 128].astype(bf16)
        for t in range(8):
            kbv_p[:, 128 * t:128 * t + 128] = kvb[128 * t:128 * t + 128, :]
        wo_p = np.empty((128, 8192), bf16)
        woT = Wo[:, 512 * g:512 * g + 512].T.astype(bf16)
        for i in range(4):
            wo_p[:, 2048 * i:2048 * i + 2048] = woT[128 * i:128 * i + 128, :]
        m = dict(
            xT=np.ascontiguousarray(hidden_states[b].T).astype(bf16),
            wq=pack_w(Wq[512 * g:512 * g + 512, :].T.astype(bf16), 4),
            wqn=pack_w(Wq_new[512 * g:512 * g + 512, :].T.astype(bf16), 4),
            wk=pack_w(Wk[128 * g:128 * g + 128, :].T.astype(bf16), 1),
            wv=pack_w(Wv[128 * g:128 * g + 128, :].T.astype(bf16), 1),
            wo=wo_p,
            kbkT=np.ascontiguousarray(
                kb_keys[b, :, 128 * g:128 * g + 128].T).astype(bf16),
            kbv=kbv_p,
            cosT=cosTs[b], sinT=sinTs[b],
            ropePT=ropePT, onesb=onesb, onesf=onesf, identf=identf,
        )
        if n_mask:
            m['masks'] = maskTs[b]
        in_maps.append(m)

    res = run_bass_kernel_spmd(nc, in_maps, core_ids=list(range(8)))
    if res.exec_time_ns is not None:
        print(f"HW exec time: {res.exec_time_ns} ns")

    out = np.zeros((B, S, H), np.float32)
    for cid in range(8):
        b = cid // 4
        out[b] += res.results[cid]["y"].astype(np.float32)
    return out
